# revision 1
# baseline (speedup 1.0000x reference)
"""Multi-head attention (B=2, S=2048, D=1024, H=16) on 8 NeuronCores.

Sharding: core c handles batch b = c//4 and head-group g = c%4 (4 heads,
F = 256 features). Data-parallel over B, tensor-parallel over heads:
Wq/Wk/Wv column-sliced, Wo row-sliced; host sums the 8 partial outputs.

Host pre-tiles every tensor into the exact SBUF layout so each DMA reads
large contiguous blocks per partition.

Device kernel (per core), everything transposed so no on-chip transposes:
  phase 1: qT/kT feature-major via fp32r matmuls, v s-major with a
           ones-column per head (softmax denominator trick), fp16.
  phase 2 (per head-pair, per sq chunk): scores^T tiles (sk, sq) via
           paired K=64 matmuls (tile_position row groups), E = exp(s/8)
           (ACT) * keep (DVE, fp16 2x mode), U' = [v,1]^T @ E accumulated
           over sk in PSUM; denominators collected per head; per-pair
           batched recip r = exp(-ln(sum)) -> ctxT = U * r (fp16).
  phase 3: out_partial = ctx @ Wo (fp16 matmuls), DMA out fp32.

Projections and scores run in float32r (tf32-class precision); the
attention-weight/value/output-projection path runs in fp16.
"""

import os

import numpy as np
import ml_dtypes

import concourse.tile as tile
from concourse import bacc, mybir
from concourse.bass_utils import run_bass_kernel_spmd

B, S, D, H = 2, 2048, 1024, 16
DH = D // H  # 64
NCORES = 8
GROUPS = 4  # head groups (cores per batch)
HL = H // GROUPS  # 4 heads per core
F = HL * DH  # 256 local features
SQC = 512  # sq chunk width
NSQ = S // SQC  # 4
SKT = S // 128  # 16 sk tiles
PD = D // 128  # 8 contraction chunks
CW = 512  # phase-1 s-chunk width
NPC = S // CW

FP32 = mybir.dt.float32
FP32R = mybir.dt.float32r
BF16 = mybir.dt.bfloat16
FP16 = mybir.dt.float16

_CACHE = {}


def _build():
    nc = bacc.Bacc("TRN2", target_bir_lowering=False, debug=False)

    xq_d = nc.dram_tensor("xqT", [NPC, 128, PD, CW], FP32R, kind="ExternalInput").ap()
    xk_d = nc.dram_tensor("xkT", [NPC, 128, PD, CW], FP32R, kind="ExternalInput").ap()
    xv_d = nc.dram_tensor("xvT", [NPC, 128, PD, CW], FP32R, kind="ExternalInput").ap()
    keep_d = nc.dram_tensor("keepT", [NSQ, 128, SKT, SQC], FP16, kind="ExternalInput").ap()
    wq_d = nc.dram_tensor("Wq", [128, PD, F], FP32R, kind="ExternalInput").ap()
    wk_d = nc.dram_tensor("Wk", [128, PD, F], FP32R, kind="ExternalInput").ap()
    wv_d = nc.dram_tensor("Wv", [128, PD, F], FP32R, kind="ExternalInput").ap()
    wo_d = nc.dram_tensor("Wo", [128, 2, D], FP16, kind="ExternalInput").ap()
    bq_d = nc.dram_tensor("bq", [F, 1], FP32, kind="ExternalInput").ap()
    bk_d = nc.dram_tensor("bk", [F, 1], FP32, kind="ExternalInput").ap()
    bv_d = nc.dram_tensor("bv", [1, F], FP32, kind="ExternalInput").ap()
    out_d = nc.dram_tensor("out", [S, D], FP32, kind="ExternalOutput").ap()

    Exp = mybir.ActivationFunctionType.Exp
    Ln = mybir.ActivationFunctionType.Ln

    with tile.TileContext(nc) as tc:
        with tc.tile_pool(name="persist", bufs=1) as pp:
            qT = pp.tile([128, 2, S], FP32R, tag="qT")  # 2 f-chunks (=head pairs)
            kT = pp.tile([128, 2, S], FP32R, tag="kT")
            v2 = pp.tile([128, SKT, HL, DH + 1], FP16, tag="v2")
            ctxT = pp.tile([128, 2, S], FP16, tag="ctxT")
            wo = pp.tile([128, 2, D], FP16, tag="wo")
            bq_sb = pp.tile([128, 2, 1], FP32, tag="bq")
            bk_sb = pp.tile([128, 2, 1], FP32, tag="bk")
            bv_bc = pp.tile([128, F], FP32, tag="bvbc")
            wu = pp.tile([128, 64], FP16, tag="wu")
            bv_row = pp.tile([1, F], FP32, tag="bvrow")

            nc.sync.dma_start(out=bq_sb[:], in_=bq_d.rearrange("(c p) o -> p c o", p=128))
            nc.sync.dma_start(out=bk_sb[:], in_=bk_d.rearrange("(c p) o -> p c o", p=128))
            nc.sync.dma_start(out=bv_row[:], in_=bv_d)
            nc.gpsimd.partition_broadcast(bv_bc[:], bv_row[:])
            nc.vector.memset(wu[:], 0.125)
            nc.vector.memset(v2[:, :, :, DH:DH + 1].bitcast(FP16), 1.0)

            # ---- phase 1: projections ----
            with tc.tile_pool(name="p1", bufs=2) as p1, \
                 tc.tile_pool(name="p1w", bufs=1) as p1w:
                wq = p1w.tile([128, PD, F], FP32R, tag="wq")
                wk = p1w.tile([128, PD, F], FP32R, tag="wk")
                wv = p1w.tile([128, PD, F], FP32R, tag="wv")
                nc.sync.dma_start(out=wq[:], in_=wq_d)
                nc.scalar.dma_start(out=wk[:], in_=wk_d)
                nc.gpsimd.dma_start(out=wv[:], in_=wv_d)

                with tc.tile_pool(name="psum1", bufs=2, space="PSUM") as ps1:
                    for s4 in range(NPC):
                        sl = slice(s4 * CW, (s4 + 1) * CW)
                        xq_sl = p1.tile([128, PD, CW], FP32R, tag="xq")
                        xk_sl = p1.tile([128, PD, CW], FP32R, tag="xk")
                        xv_sl = p1.tile([128, PD, CW], FP32R, tag="xv")
                        nc.sync.dma_start(out=xq_sl[:], in_=xq_d[s4])
                        nc.scalar.dma_start(out=xk_sl[:], in_=xk_d[s4])
                        nc.gpsimd.dma_start(out=xv_sl[:], in_=xv_d[s4])

                        for fc in range(2):
                            fsl = slice(fc * 128, (fc + 1) * 128)
                            q_ps = ps1.tile([128, CW], FP32, tag="q_ps")
                            for d in range(PD):
                                nc.tensor.matmul(q_ps[:], wq[:, d, fsl], xq_sl[:, d, :],
                                                 start=(d == 0), stop=(d == PD - 1))
                            nc.scalar.add(qT[:, fc, sl], q_ps[:], bq_sb[:, fc, :])

                            k_ps = ps1.tile([128, CW], FP32, tag="k_ps")
                            for d in range(PD):
                                nc.tensor.matmul(k_ps[:], wk[:, d, fsl], xk_sl[:, d, :],
                                                 start=(d == 0), stop=(d == PD - 1))
                            nc.scalar.add(kT[:, fc, sl], k_ps[:], bk_sb[:, fc, :])

                        for m in range(CW // 128):  # s-subtiles of 128
                            ti = s4 * (CW // 128) + m
                            msl = slice(m * 128, (m + 1) * 128)
                            v_ps = ps1.tile([128, F], FP32, tag="v_ps")
                            for d in range(PD):
                                nc.tensor.matmul(v_ps[:], xv_sl[:, d, msl], wv[:, d, :],
                                                 start=(d == 0), stop=(d == PD - 1))
                            for hh in range(HL):
                                hsl = slice(hh * DH, (hh + 1) * DH)
                                nc.vector.tensor_add(
                                    v2[:, ti, hh, 0:DH],
                                    v_ps[:, hsl],
                                    bv_bc[:, hsl],
                                )

            nc.sync.dma_start(out=wo[:], in_=wo_d)

            # ---- phase 2: attention (head pair outer, sq inner) ----
            with tc.tile_pool(name="p2", bufs=2) as p2, \
                 tc.tile_pool(name="p2e", bufs=6) as p2e, \
                 tc.tile_pool(name="rec", bufs=2) as rec, \
                 tc.tile_pool(name="psum_st", bufs=2, space="PSUM") as ps_st, \
                 tc.tile_pool(name="psum_u", bufs=4, space="PSUM") as ps_u:
                sums = [rec.tile([1, S], FP32, tag=f"sums{h}", name=f"sums{h}", bufs=1)
                        for h in range(HL)]
                wu_ps = ps_st.tile([128, 2, SQC], FP32, tag="st", name="wu_ps")
                for i in range(40):
                    nc.tensor.matmul(wu_ps[0:64, 0, 0:64], wu[:], wu[:],
                                     start=True, stop=True)
                def issue_keep(hp, sq):
                    t = p2.tile([128, SKT, SQC], FP16, tag="keep",
                                name=f"keep_{hp}_{sq}", bufs=3)
                    nc.gpsimd.dma_start(out=t[:], in_=keep_d[sq])
                    return t

                keep_next = issue_keep(0, 0)
                for hp in range(2):
                    for sq in range(NSQ):
                        qsl = slice(sq * SQC, (sq + 1) * SQC)
                        keep = keep_next
                        if (hp, sq) != (1, NSQ - 1):
                            nsq = (sq + 1) % NSQ
                            keep_next = issue_keep(hp + (1 if nsq == 0 else 0), nsq)
                        u = [ps_u.tile([128, 1, SQC], FP32, tag="u", name=f"u_{hp}_{sq}_{j}")
                             for j in range(2)]
                        for sk in range(SKT):
                            ksl = slice(sk * 128, (sk + 1) * 128)
                            st_ps = ps_st.tile([128, 2, SQC], FP32, tag="st")
                            nc.tensor.matmul(st_ps[:, 0, :], kT[0:64, hp, ksl],
                                             qT[0:64, hp, qsl], start=True, stop=True,
                                             tile_position=(0, 0))
                            nc.tensor.matmul(st_ps[:, 1, :], kT[64:128, hp, ksl],
                                             qT[64:128, hp, qsl], start=True, stop=True,
                                             tile_position=(64, 0))
                            e_sb = p2e.tile([128, 2, SQC], FP16, tag="e")
                            nc.scalar.activation(e_sb[:], st_ps[:], Exp, scale=0.125)
                            e2 = p2e.tile([128, 2, SQC], FP16, tag="e2")
                            nc.vector.tensor_mul(e2[:, 0, :], e_sb[:, 0, :], keep[:, sk, :])
                            nc.vector.tensor_mul(e2[:, 1, :], e_sb[:, 1, :], keep[:, sk, :])
                            for j in range(2):
                                nc.tensor.matmul(
                                    u[j][0:DH + 1, 0, :],
                                    v2[:, sk, 2 * hp + j, :],
                                    e2[:, j, :],
                                    start=(sk == 0), stop=(sk == SKT - 1),
                                )
                        for j in range(2):
                            nc.vector.tensor_copy(sums[2 * hp + j][0:1, qsl],
                                                  u[j][DH:DH + 1, 0, :])
                            nc.scalar.copy(ctxT[j * DH:(j + 1) * DH, hp, qsl],
                                           u[j][0:DH, 0, :])
                    # per-pair batched reciprocal r = exp(-ln(sum))
                    lns = [rec.tile([1, S], FP32, tag=f"lns{j}", name=f"lns{hp}_{j}", bufs=1)
                           for j in range(2)]
                    with tc.tile_critical():
                        for j in range(2):
                            nc.scalar.activation(lns[j][:], sums[2 * hp + j][:], Ln)
                        for j in range(2):
                            nc.scalar.activation(sums[2 * hp + j][:], lns[j][:], Exp,
                                                 scale=-1.0)
                    for j in range(2):
                        rb = rec.tile([128, S], FP32, tag="rb")
                        nc.gpsimd.partition_broadcast(rb[:], sums[2 * hp + j][:])
                        nc.vector.tensor_mul(ctxT[j * DH:(j + 1) * DH, hp, :],
                                             ctxT[j * DH:(j + 1) * DH, hp, :],
                                             rb[j * DH:(j + 1) * DH, :])

            # ---- phase 3: output projection ----
            with tc.tile_pool(name="p3", bufs=4) as p3, \
                 tc.tile_pool(name="psum3", bufs=4, space="PSUM") as ps3:
                wu3_ps = ps3.tile([128, 512], FP32, tag="o_ps", name="wu3_ps")
                for i in range(40):
                    nc.tensor.matmul(wu3_ps[0:64, 0:64], wu[:], wu[:],
                                     start=True, stop=True)
                for ti in range(S // 128):
                    tsl = slice(ti * 128, (ti + 1) * 128)
                    o_sb = p3.tile([128, 2, 512], FP32, tag="o_sb")
                    for n in range(2):
                        nsl = slice(n * 512, (n + 1) * 512)
                        o_ps = ps3.tile([128, 512], FP32, tag="o_ps")
                        for fc in range(2):
                            nc.tensor.matmul(o_ps[:], ctxT[:, fc, tsl], wo[:, fc, nsl],
                                             start=(fc == 0), stop=(fc == 1))
                        if n == 0:
                            nc.vector.tensor_copy(o_sb[:, n, :], o_ps[:])
                        else:
                            nc.scalar.copy(o_sb[:, n, :], o_ps[:])
                    eng = nc.sync if ti % 2 == 0 else nc.scalar
                    eng.dma_start(out=out_d[tsl, :], in_=o_sb[:].rearrange("p a b -> p (a b)"))

    nc.compile()
    return nc


def _tile_x(xT):
    # (D, S) -> (NPC, 128, PD, CW); [s4, p, c, j] = xT[c*128+p, s4*CW+j]
    return np.ascontiguousarray(
        xT.reshape(PD, 128, NPC, CW).transpose(2, 1, 0, 3))


def kernel(query, key, value, mask, Wq, bq, Wk, bk, Wv, bv, Wo, bo, **_):
    if "nc" not in _CACHE:
        _CACHE["nc"] = _build()
    nc = _CACHE["nc"]

    query = np.asarray(query, np.float32)
    key = np.asarray(key, np.float32)
    value = np.asarray(value, np.float32)
    mask = np.asarray(mask)
    Wq = np.asarray(Wq, np.float32)
    Wk = np.asarray(Wk, np.float32)
    Wv = np.asarray(Wv, np.float32)
    Wo = np.asarray(Wo, np.float32)
    bq = np.asarray(bq, np.float32)
    bk = np.asarray(bk, np.float32)
    bv = np.asarray(bv, np.float32)
    bo = np.asarray(bo, np.float32)

    xT = {}
    keepT = {}
    for b in range(B):
        xT[b] = (
            _tile_x(query[b].T),
            _tile_x(key[b].T),
            _tile_x(value[b].T),
        )
        kp = (~mask[b]).T.astype(np.float16)  # (sk, sq)
        keepT[b] = np.ascontiguousarray(
            kp.reshape(SKT, 128, NSQ, SQC).transpose(2, 1, 0, 3))

    wsl = {}
    for g in range(GROUPS):
        fs = slice(g * F, (g + 1) * F)
        wsl[g] = (
            np.ascontiguousarray(Wq[:, fs].reshape(PD, 128, F).transpose(1, 0, 2)),
            np.ascontiguousarray(Wk[:, fs].reshape(PD, 128, F).transpose(1, 0, 2)),
            np.ascontiguousarray(Wv[:, fs].reshape(PD, 128, F).transpose(1, 0, 2)),
            np.ascontiguousarray(
                Wo[fs, :].astype(np.float16).reshape(2, 128, D).transpose(1, 0, 2)),
            np.ascontiguousarray(bq[fs].reshape(F, 1)),
            np.ascontiguousarray(bk[fs].reshape(F, 1)),
            np.ascontiguousarray(bv[fs].reshape(1, F)),
        )

    in_maps = []
    for c in range(NCORES):
        b, g = c // GROUPS, c % GROUPS
        wq_s, wk_s, wv_s, wo_s, bq_s, bk_s, bv_s = wsl[g]
        in_maps.append({
            "xqT": xT[b][0], "xkT": xT[b][1], "xvT": xT[b][2],
            "keepT": keepT[b],
            "Wq": wq_s, "Wk": wk_s, "Wv": wv_s, "Wo": wo_s,
            "bq": bq_s, "bk": bk_s, "bv": bv_s,
        })

    res = run_bass_kernel_spmd(nc, in_maps, core_ids=list(range(NCORES)))
    outs = [r["out"] for r in res.results]
    full = np.empty((B, S, D), np.float32)
    for b in range(B):
        acc = outs[GROUPS * b].astype(np.float32)
        for g in range(1, GROUPS):
            acc = acc + outs[GROUPS * b + g]
        full[b] = acc + bo
    return full



# revision 2
# speedup vs baseline: 1.1524x; 1.1524x over previous
"""Multi-head attention (B=2, S=2048, D=1024, H=16) on 8 NeuronCores.

Sharding: core c handles batch b = c//4 and head-group g = c%4 (4 heads,
F = 256 features). Data-parallel over B, tensor-parallel over heads:
Wq/Wk/Wv column-sliced, Wo row-sliced; host sums the 8 partial outputs.

Host pre-tiles every tensor into the exact SBUF layout so each DMA reads
large contiguous blocks per partition.

Device kernel (per core), everything transposed so no on-chip transposes:
  phase 1: qT/kT feature-major bf16 matmuls (bias fused into the PSUM
           evacuation via tensor_scalar), v s-major with a ones-column
           per head (softmax denominator rides in the AV matmul), fp16.
  phase 2 (sq-chunk outer, head-pair inner): scores^T tiles (sk, sq) via
           paired K=64 bf16 matmuls (tile_position row groups), E =
           exp(s/8) (ACT, fp16 out) * keep (DVE fp16 2x), U' = [v,1]^T @ E
           accumulated over sk in PSUM. Per chunk: denominators ->
           reciprocal_approx_fast (DVE), partition_broadcast + ctx
           normalize on GpSimd, then the output projection for that chunk
           streams out immediately (fp16 partials; host sums in fp32).
"""

import numpy as np
import ml_dtypes

import concourse.tile as tile
from concourse import bacc, mybir
from concourse.bass_utils import run_bass_kernel_spmd

B, S, D, H = 2, 2048, 1024, 16
DH = D // H  # 64
NCORES = 8
GROUPS = 4  # head groups (cores per batch)
HL = H // GROUPS  # 4 heads per core
F = HL * DH  # 256 local features
SQC = 512  # sq chunk width
NSQ = S // SQC  # 4
SKT = S // 128  # 16 sk tiles
PD = D // 128  # 8 contraction chunks
CW = 512  # phase-1 s-chunk width
NPC = S // CW

FP32 = mybir.dt.float32
BF16 = mybir.dt.bfloat16
FP16 = mybir.dt.float16

MULT = mybir.AluOpType.mult
ADD = mybir.AluOpType.add

_CACHE = {}


def _build():
    nc = bacc.Bacc("TRN2", target_bir_lowering=False, debug=False)

    xq_d = nc.dram_tensor("xqT", [NPC, 128, PD, CW], BF16, kind="ExternalInput").ap()
    xk_d = nc.dram_tensor("xkT", [NPC, 128, PD, CW], BF16, kind="ExternalInput").ap()
    xv_d = nc.dram_tensor("xvT", [NPC, 128, PD, CW], BF16, kind="ExternalInput").ap()
    keep_d = nc.dram_tensor("keepT", [NSQ, 128, SKT, SQC], FP16, kind="ExternalInput").ap()
    wq_d = nc.dram_tensor("Wq", [128, PD, F], BF16, kind="ExternalInput").ap()
    wk_d = nc.dram_tensor("Wk", [128, PD, F], BF16, kind="ExternalInput").ap()
    wv_d = nc.dram_tensor("Wv", [128, PD, F], BF16, kind="ExternalInput").ap()
    wo_d = nc.dram_tensor("Wo", [128, 2, D], FP16, kind="ExternalInput").ap()
    bqk_d = nc.dram_tensor("bqk", [128, 2, 2, 1], FP32, kind="ExternalInput").ap()
    bv_d = nc.dram_tensor("bv", [1, F], FP32, kind="ExternalInput").ap()
    out_d = nc.dram_tensor("out", [S, D], FP16, kind="ExternalOutput").ap()

    Exp = mybir.ActivationFunctionType.Exp

    with tile.TileContext(nc) as tc:
        with tc.tile_pool(name="persist", bufs=1) as pp, \
             tc.tile_pool(name="p2k", bufs=2) as p2k:
            qT = pp.tile([128, 2, S], BF16, tag="qT")  # 2 f-chunks (=head pairs)
            kT = pp.tile([128, 2, S], BF16, tag="kT")
            v2 = pp.tile([128, SKT, HL, DH + 1], FP16, tag="v2")
            ctxT = pp.tile([128, 2, S], FP16, tag="ctxT")
            wo = pp.tile([128, 2, D], FP16, tag="wo")
            bqk = pp.tile([128, 2, 2, 1], FP32, tag="bqk")
            bv_row = pp.tile([1, F], FP32, tag="bvrow")
            bv_bc = pp.tile([128, F], FP32, tag="bvbc")

            def issue_keep(sq):
                t = p2k.tile([128, SKT, SQC], FP16, tag="keep", name=f"keep_{sq}")
                nc.gpsimd.dma_start(out=t[:], in_=keep_d[sq])
                return t

            nc.sync.dma_start(out=bqk[:], in_=bqk_d)
            nc.sync.dma_start(out=bv_row[:], in_=bv_d)
            nc.scalar.dma_start(out=wo[:], in_=wo_d)
            nc.gpsimd.partition_broadcast(bv_bc[:], bv_row[:])
            nc.vector.memset(v2[:, :, :, DH:DH + 1], 1.0)
            keep_bufs = [issue_keep(0), issue_keep(1)]

            # ---- phase 1: projections ----
            with tc.tile_pool(name="p1w", bufs=1) as p1w, \
                 tc.tile_pool(name="p1x", bufs=2) as p1x, \
                 tc.tile_pool(name="ps1", bufs=2, space="PSUM") as ps1:
                wq = p1w.tile([128, PD, F], BF16, tag="wq")
                wk = p1w.tile([128, PD, F], BF16, tag="wk")
                wv = p1w.tile([128, PD, F], BF16, tag="wv")
                nc.sync.dma_start(out=wq[:], in_=wq_d)
                nc.scalar.dma_start(out=wk[:], in_=wk_d)
                nc.gpsimd.dma_start(out=wv[:], in_=wv_d)

                for s4 in range(NPC):
                    sl = slice(s4 * CW, (s4 + 1) * CW)
                    xq_sl = p1x.tile([128, PD, CW], BF16, tag="xq")
                    xk_sl = p1x.tile([128, PD, CW], BF16, tag="xk")
                    xv_sl = p1x.tile([128, PD, CW], BF16, tag="xv")
                    nc.sync.dma_start(out=xq_sl[:], in_=xq_d[s4])
                    nc.scalar.dma_start(out=xk_sl[:], in_=xk_d[s4])
                    nc.gpsimd.dma_start(out=xv_sl[:], in_=xv_d[s4])

                    for fc in range(2):
                        fsl = slice(fc * 128, (fc + 1) * 128)
                        q_ps = ps1.tile([128, CW], FP32, tag="q_ps")
                        for d in range(PD):
                            nc.tensor.matmul(q_ps[:], wq[:, d, fsl], xq_sl[:, d, :],
                                             start=(d == 0), stop=(d == PD - 1))
                        nc.vector.tensor_scalar_add(qT[:, fc, sl], q_ps[:],
                                                    bqk[:, 0, fc, :])

                        k_ps = ps1.tile([128, CW], FP32, tag="k_ps")
                        for d in range(PD):
                            nc.tensor.matmul(k_ps[:], wk[:, d, fsl], xk_sl[:, d, :],
                                             start=(d == 0), stop=(d == PD - 1))
                        nc.vector.tensor_scalar_add(kT[:, fc, sl], k_ps[:],
                                                    bqk[:, 1, fc, :])

                    for m in range(CW // 128):  # s-subtiles of 128
                        ti = s4 * (CW // 128) + m
                        msl = slice(m * 128, (m + 1) * 128)
                        v_ps = ps1.tile([128, F], FP32, tag="v_ps")
                        for d in range(PD):
                            nc.tensor.matmul(v_ps[:], xv_sl[:, d, msl], wv[:, d, :],
                                             start=(d == 0), stop=(d == PD - 1))
                        nc.vector.scalar_tensor_tensor(
                            out=v2[:, ti, :, 0:DH],
                            in0=v_ps.rearrange("p (h e) -> p h e", h=HL),
                            scalar=1.0,
                            in1=bv_bc.rearrange("p (h e) -> p h e", h=HL),
                            op0=MULT, op1=ADD,
                        )

            # ---- phase 2+3: attention + output projection, per sq chunk ----
            with tc.tile_pool(name="p2e", bufs=3) as p2e, \
                 tc.tile_pool(name="p2s", bufs=2) as p2s, \
                 tc.tile_pool(name="p3o", bufs=3) as p3o, \
                 tc.tile_pool(name="ps_st", bufs=2, space="PSUM") as ps_st, \
                 tc.tile_pool(name="ps_u", bufs=1, space="PSUM") as ps_u, \
                 tc.tile_pool(name="ps_o", bufs=2, space="PSUM") as ps_o:
                for sq in range(NSQ):
                    qsl = slice(sq * SQC, (sq + 1) * SQC)
                    keep = keep_bufs[sq % 2]
                    if sq + 2 < NSQ:
                        keep_bufs[sq % 2] = issue_keep(sq + 2)
                    sums = p2s.tile([1, HL, SQC], FP32, tag="sums")
                    for hp in range(2):
                        u = [ps_u.tile([128, SQC], FP32, tag=f"u{j}",
                                       name=f"u_{sq}_{hp}_{j}")
                             for j in range(2)]
                        for sk in range(SKT):
                            ksl = slice(sk * 128, (sk + 1) * 128)
                            st = ps_st.tile([128, 2, SQC], FP32, tag="st")
                            nc.tensor.matmul(st[:, 0, :], kT[0:64, hp, ksl],
                                             qT[0:64, hp, qsl], start=True, stop=True,
                                             tile_position=(0, 0))
                            nc.tensor.matmul(st[:, 1, :], kT[64:128, hp, ksl],
                                             qT[64:128, hp, qsl], start=True, stop=True,
                                             tile_position=(64, 0))
                            e_sb = p2e.tile([128, 2, SQC], FP16, tag="e")
                            nc.scalar.activation(e_sb[:], st[:], Exp, scale=0.125)
                            e2 = p2e.tile([128, 2, SQC], FP16, tag="e2")
                            nc.vector.tensor_mul(e2[:, 0, :], e_sb[:, 0, :],
                                                 keep[:, sk, :])
                            nc.vector.tensor_mul(e2[:, 1, :], e_sb[:, 1, :],
                                                 keep[:, sk, :])
                            for j in range(2):
                                nc.tensor.matmul(
                                    u[j][0:DH + 1, :],
                                    v2[:, sk, 2 * hp + j, :],
                                    e2[:, j, :],
                                    start=(sk == 0), stop=(sk == SKT - 1),
                                )
                        for j in range(2):
                            nc.vector.tensor_copy(sums[0:1, 2 * hp + j, :],
                                                  u[j][DH:DH + 1, :])
                            nc.vector.tensor_copy(ctxT[j * DH:(j + 1) * DH, hp, qsl],
                                                  u[j][0:DH, :])
                    # normalize: r = 1/sums, broadcast, scale ctx (off ACT)
                    r = p2s.tile([1, HL, SQC], FP32, tag="r")
                    nc.vector.reciprocal_approx_fast(r[:], sums[:])
                    rb = p2s.tile([128, HL, SQC], FP32, tag="rb")
                    nc.gpsimd.partition_broadcast(rb[:], r[:])
                    for hp in range(2):
                        for j in range(2):
                            nc.gpsimd.tensor_mul(
                                ctxT[j * DH:(j + 1) * DH, hp, qsl],
                                ctxT[j * DH:(j + 1) * DH, hp, qsl],
                                rb[j * DH:(j + 1) * DH, 2 * hp + j, :])
                    # output projection for this chunk
                    for m in range(SQC // 128):
                        ti = sq * (SQC // 128) + m
                        tsl = slice(ti * 128, (ti + 1) * 128)
                        o_sb = p3o.tile([128, 2, 512], FP16, tag="o_sb")
                        for n in range(2):
                            nsl = slice(n * 512, (n + 1) * 512)
                            o_ps = ps_o.tile([128, 512], FP32, tag="o_ps")
                            for fc in range(2):
                                nc.tensor.matmul(o_ps[:], ctxT[:, fc, tsl],
                                                 wo[:, fc, nsl],
                                                 start=(fc == 0), stop=(fc == 1))
                            if n == 0:
                                nc.vector.tensor_copy(o_sb[:, n, :], o_ps[:])
                            else:
                                nc.scalar.copy(o_sb[:, n, :], o_ps[:])
                        eng = nc.sync if m % 2 == 0 else nc.scalar
                        eng.dma_start(out=out_d[tsl, :],
                                      in_=o_sb[:].rearrange("p a b -> p (a b)"))

    nc.compile()
    return nc


def _tile_x(xT):
    # (D, S) -> (NPC, 128, PD, CW); [s4, p, c, j] = xT[c*128+p, s4*CW+j]
    return np.ascontiguousarray(
        xT.reshape(PD, 128, NPC, CW).transpose(2, 1, 0, 3).astype(ml_dtypes.bfloat16))


def kernel(query, key, value, mask, Wq, bq, Wk, bk, Wv, bv, Wo, bo, **_):
    if "nc" not in _CACHE:
        _CACHE["nc"] = _build()
    nc = _CACHE["nc"]

    query = np.asarray(query, np.float32)
    key = np.asarray(key, np.float32)
    value = np.asarray(value, np.float32)
    mask = np.asarray(mask)
    Wq = np.asarray(Wq, np.float32)
    Wk = np.asarray(Wk, np.float32)
    Wv = np.asarray(Wv, np.float32)
    Wo = np.asarray(Wo, np.float32)
    bq = np.asarray(bq, np.float32)
    bk = np.asarray(bk, np.float32)
    bv = np.asarray(bv, np.float32)
    bo = np.asarray(bo, np.float32)

    xT = {}
    keepT = {}
    for b in range(B):
        xT[b] = (
            _tile_x(query[b].T),
            _tile_x(key[b].T),
            _tile_x(value[b].T),
        )
        kp = (~mask[b]).T.astype(np.float16)  # (sk, sq)
        keepT[b] = np.ascontiguousarray(
            kp.reshape(SKT, 128, NSQ, SQC).transpose(2, 1, 0, 3))

    wsl = {}
    for g in range(GROUPS):
        fs = slice(g * F, (g + 1) * F)
        bq2 = bq[fs].reshape(2, 128).T  # [p, fc]
        bk2 = bk[fs].reshape(2, 128).T
        wsl[g] = (
            np.ascontiguousarray(
                Wq[:, fs].reshape(PD, 128, F).transpose(1, 0, 2)
            ).astype(ml_dtypes.bfloat16),
            np.ascontiguousarray(
                Wk[:, fs].reshape(PD, 128, F).transpose(1, 0, 2)
            ).astype(ml_dtypes.bfloat16),
            np.ascontiguousarray(
                Wv[:, fs].reshape(PD, 128, F).transpose(1, 0, 2)
            ).astype(ml_dtypes.bfloat16),
            np.ascontiguousarray(
                Wo[fs, :].astype(np.float16).reshape(2, 128, D).transpose(1, 0, 2)),
            np.ascontiguousarray(
                np.stack([bq2, bk2], axis=1)[:, :, :, None].astype(np.float32)),
            np.ascontiguousarray(bv[fs].reshape(1, F)),
        )

    in_maps = []
    for c in range(NCORES):
        b, g = c // GROUPS, c % GROUPS
        wq_s, wk_s, wv_s, wo_s, bqk_s, bv_s = wsl[g]
        in_maps.append({
            "xqT": xT[b][0], "xkT": xT[b][1], "xvT": xT[b][2],
            "keepT": keepT[b],
            "Wq": wq_s, "Wk": wk_s, "Wv": wv_s, "Wo": wo_s,
            "bqk": bqk_s, "bv": bv_s,
        })

    res = run_bass_kernel_spmd(nc, in_maps, core_ids=list(range(NCORES)))
    outs = [r["out"] for r in res.results]
    full = np.empty((B, S, D), np.float32)
    for b in range(B):
        acc = outs[GROUPS * b].astype(np.float32)
        for g in range(1, GROUPS):
            acc = acc + outs[GROUPS * b + g]
        full[b] = acc + bo
    return full


# revision 3
# speedup vs baseline: 1.2749x; 1.1063x over previous
"""Multi-head attention (B=2, S=2048, D=1024, H=16) on 8 NeuronCores.

Sharding: core c handles batch b = c//4 and head-group g = c%4 (4 heads,
F = 256 features). Data-parallel over B, tensor-parallel over heads:
Wq/Wk/Wv column-sliced, Wo row-sliced; host sums the 8 partial outputs.

Host pre-tiles every tensor into the exact SBUF layout so each DMA reads
large contiguous blocks per partition.

Device kernel (per core), everything transposed so no on-chip transposes:
  phase 1: qT/kT feature-major bf16 matmuls (bias fused into the PSUM
           evacuation via tensor_scalar), v s-major with a ones-column
           per head (softmax denominator rides in the AV matmul), fp16.
  phase 2 (sq-chunk outer, head-pair inner): scores^T tiles (sk, sq) via
           paired K=64 bf16 matmuls (tile_position row groups), E =
           exp(s/8) (ACT, fp16 out) * keep (DVE fp16 2x), U' = [v,1]^T @ E
           accumulated over sk in PSUM. Per chunk: denominators ->
           reciprocal_approx_fast (DVE), partition_broadcast + ctx
           normalize on GpSimd, then the output projection for that chunk
           streams out immediately (fp16 partials; host sums in fp32).
"""

import numpy as np
import ml_dtypes

import concourse.tile as tile
from concourse import bacc, mybir
from concourse.bass_utils import run_bass_kernel_spmd

B, S, D, H = 2, 2048, 1024, 16
DH = D // H  # 64
NCORES = 8
GROUPS = 4  # head groups (cores per batch)
HL = H // GROUPS  # 4 heads per core
F = HL * DH  # 256 local features
SQC = 512  # sq chunk width
NSQ = S // SQC  # 4
SKT = S // 128  # 16 sk tiles
PD = D // 128  # 8 contraction chunks
CW = 512  # phase-1 s-chunk width
NPC = S // CW

FP32 = mybir.dt.float32
BF16 = mybir.dt.bfloat16
FP16 = mybir.dt.float16

MULT = mybir.AluOpType.mult
ADD = mybir.AluOpType.add

_CACHE = {}


def _build():
    nc = bacc.Bacc("TRN2", target_bir_lowering=False, debug=False)

    xq_d = nc.dram_tensor("xqT", [NPC, 128, PD, CW], BF16, kind="ExternalInput").ap()
    xk_d = nc.dram_tensor("xkT", [NPC, 128, PD, CW], BF16, kind="ExternalInput").ap()
    xv_d = nc.dram_tensor("xvT", [NPC, 128, PD, CW], BF16, kind="ExternalInput").ap()
    keep_d = nc.dram_tensor("keepT", [NSQ, 128, SKT, SQC], FP16, kind="ExternalInput").ap()
    wq_d = nc.dram_tensor("Wq", [128, PD, F], BF16, kind="ExternalInput").ap()
    wk_d = nc.dram_tensor("Wk", [128, PD, F], BF16, kind="ExternalInput").ap()
    wv_d = nc.dram_tensor("Wv", [128, PD, F], BF16, kind="ExternalInput").ap()
    wo_d = nc.dram_tensor("Wo", [128, 2, D], FP16, kind="ExternalInput").ap()
    bqk_d = nc.dram_tensor("bqk", [128, 2, 2, 1], FP32, kind="ExternalInput").ap()
    bv_d = nc.dram_tensor("bv", [1, F], FP32, kind="ExternalInput").ap()
    out_d = nc.dram_tensor("out", [S, D], FP16, kind="ExternalOutput").ap()

    Exp = mybir.ActivationFunctionType.Exp

    with tile.TileContext(nc) as tc:
        with tc.tile_pool(name="persist", bufs=1) as pp, \
             tc.tile_pool(name="p2k", bufs=2) as p2k:
            qT = pp.tile([128, 2, S], BF16, tag="qT")  # 2 f-chunks (=head pairs)
            kT = pp.tile([128, 2, S], BF16, tag="kT")
            v2 = pp.tile([128, SKT, HL, DH + 1], FP16, tag="v2")
            ctxT = pp.tile([128, 2, S], FP16, tag="ctxT")
            wo = pp.tile([128, 2, D], FP16, tag="wo")
            bqk = pp.tile([128, 2, 2, 1], FP32, tag="bqk")
            bv_row = pp.tile([1, F], FP32, tag="bvrow")
            bv_bc = pp.tile([128, F], FP32, tag="bvbc")

            def issue_keep(sq):
                t = p2k.tile([128, SKT, SQC], FP16, tag="keep", name=f"keep_{sq}")
                nc.gpsimd.dma_start(out=t[:], in_=keep_d[sq])
                return t

            nc.sync.dma_start(out=bqk[:], in_=bqk_d)
            nc.sync.dma_start(out=bv_row[:], in_=bv_d)
            nc.scalar.dma_start(out=wo[:], in_=wo_d)
            nc.gpsimd.partition_broadcast(bv_bc[:], bv_row[:])
            nc.vector.memset(v2[:, :, :, DH:DH + 1], 1.0)
            keep_bufs = [issue_keep(0), issue_keep(1)]

            # ---- phase 1: projections ----
            with tc.tile_pool(name="p1w", bufs=1) as p1w, \
                 tc.tile_pool(name="p1x", bufs=2) as p1x, \
                 tc.tile_pool(name="ps1", bufs=2, space="PSUM") as ps1:
                wq = p1w.tile([128, PD, F], BF16, tag="wq")
                wk = p1w.tile([128, PD, F], BF16, tag="wk")
                wv = p1w.tile([128, PD, F], BF16, tag="wv")
                nc.sync.dma_start(out=wq[:], in_=wq_d)
                nc.scalar.dma_start(out=wk[:], in_=wk_d)
                nc.gpsimd.dma_start(out=wv[:], in_=wv_d)

                for s4 in range(NPC):
                    sl = slice(s4 * CW, (s4 + 1) * CW)
                    xq_sl = p1x.tile([128, PD, CW], BF16, tag="xq")
                    xk_sl = p1x.tile([128, PD, CW], BF16, tag="xk")
                    xv_sl = p1x.tile([128, PD, CW], BF16, tag="xv")
                    nc.sync.dma_start(out=xq_sl[:], in_=xq_d[s4])
                    nc.scalar.dma_start(out=xk_sl[:], in_=xk_d[s4])
                    nc.gpsimd.dma_start(out=xv_sl[:], in_=xv_d[s4])

                    for fc in range(2):
                        fsl = slice(fc * 128, (fc + 1) * 128)
                        q_ps = ps1.tile([128, CW], FP32, tag="q_ps")
                        for d in range(PD):
                            nc.tensor.matmul(q_ps[:], wq[:, d, fsl], xq_sl[:, d, :],
                                             start=(d == 0), stop=(d == PD - 1))
                        nc.vector.tensor_scalar_add(qT[:, fc, sl], q_ps[:],
                                                    bqk[:, 0, fc, :])

                        k_ps = ps1.tile([128, CW], FP32, tag="k_ps")
                        for d in range(PD):
                            nc.tensor.matmul(k_ps[:], wk[:, d, fsl], xk_sl[:, d, :],
                                             start=(d == 0), stop=(d == PD - 1))
                        nc.vector.tensor_scalar_add(kT[:, fc, sl], k_ps[:],
                                                    bqk[:, 1, fc, :])

                    for m in range(CW // 128):  # s-subtiles of 128
                        ti = s4 * (CW // 128) + m
                        msl = slice(m * 128, (m + 1) * 128)
                        v_ps = ps1.tile([128, F], FP32, tag="v_ps")
                        for d in range(PD):
                            nc.tensor.matmul(v_ps[:], xv_sl[:, d, msl], wv[:, d, :],
                                             start=(d == 0), stop=(d == PD - 1))
                        nc.vector.scalar_tensor_tensor(
                            out=v2[:, ti, :, 0:DH],
                            in0=v_ps.rearrange("p (h e) -> p h e", h=HL),
                            scalar=1.0,
                            in1=bv_bc.rearrange("p (h e) -> p h e", h=HL),
                            op0=MULT, op1=ADD,
                        )

            # ---- phase 2+3: attention + output projection, per sq chunk ----
            # Phase 3 for chunk c is emitted inside chunk c+1 (after its hp=0
            # pass) so the in-order engine queues never stall on the
            # normalize chain at a chunk boundary.
            with tc.tile_pool(name="p2e", bufs=6) as p2e, \
                 tc.tile_pool(name="p2s", bufs=2) as p2s, \
                 tc.tile_pool(name="p3o", bufs=3) as p3o, \
                 tc.tile_pool(name="ps_st", bufs=2, space="PSUM") as ps_st, \
                 tc.tile_pool(name="ps_u", bufs=1, space="PSUM") as ps_u, \
                 tc.tile_pool(name="ps_o", bufs=2, space="PSUM") as ps_o:

                def phase3(sq):
                    for m in range(SQC // 128):
                        ti = sq * (SQC // 128) + m
                        tsl = slice(ti * 128, (ti + 1) * 128)
                        o_sb = p3o.tile([128, 2, 512], FP16, tag="o_sb",
                                        name=f"o_sb_{ti}")
                        for n in range(2):
                            nsl = slice(n * 512, (n + 1) * 512)
                            o_ps = ps_o.tile([128, 512], FP32, tag="o_ps",
                                             name=f"o_ps_{ti}_{n}")
                            for fc in range(2):
                                nc.tensor.matmul(o_ps[:], ctxT[:, fc, tsl],
                                                 wo[:, fc, nsl],
                                                 start=(fc == 0), stop=(fc == 1))
                            if n == 0:
                                nc.vector.tensor_copy(o_sb[:, n, :], o_ps[:])
                            else:
                                nc.scalar.copy(o_sb[:, n, :], o_ps[:])
                        eng = nc.sync if m % 2 == 0 else nc.scalar
                        eng.dma_start(out=out_d[tsl, :],
                                      in_=o_sb[:].rearrange("p a b -> p (a b)"))

                for sq in range(NSQ):
                    qsl = slice(sq * SQC, (sq + 1) * SQC)
                    keep = keep_bufs[sq % 2]
                    if sq + 2 < NSQ:
                        keep_bufs[sq % 2] = issue_keep(sq + 2)
                    sums = p2s.tile([1, HL, SQC], FP32, tag="sums")
                    for hp in range(2):
                        u = [ps_u.tile([128, SQC], FP32, tag=f"u{j}",
                                       name=f"u_{sq}_{hp}_{j}")
                             for j in range(2)]
                        for sk in range(SKT):
                            ksl = slice(sk * 128, (sk + 1) * 128)
                            st = ps_st.tile([128, 2, SQC], FP32, tag="st")
                            nc.tensor.matmul(st[:, 0, :], kT[0:64, hp, ksl],
                                             qT[0:64, hp, qsl], start=True, stop=True,
                                             tile_position=(0, 0))
                            nc.tensor.matmul(st[:, 1, :], kT[64:128, hp, ksl],
                                             qT[64:128, hp, qsl], start=True, stop=True,
                                             tile_position=(64, 0))
                            e_sb = p2e.tile([128, 2, SQC], FP16, tag="e")
                            nc.scalar.activation(e_sb[:], st[:], Exp, scale=0.125)
                            e2 = p2e.tile([128, 2, SQC], FP16, tag="e2")
                            nc.vector.tensor_mul(e2[:, 0, :], e_sb[:, 0, :],
                                                 keep[:, sk, :])
                            nc.vector.tensor_mul(e2[:, 1, :], e_sb[:, 1, :],
                                                 keep[:, sk, :])
                            for j in range(2):
                                nc.tensor.matmul(
                                    u[j][0:DH + 1, :],
                                    v2[:, sk, 2 * hp + j, :],
                                    e2[:, j, :],
                                    start=(sk == 0), stop=(sk == SKT - 1),
                                )
                        for j in range(2):
                            nc.vector.tensor_copy(sums[0:1, 2 * hp + j, :],
                                                  u[j][DH:DH + 1, :])
                            nc.vector.tensor_copy(ctxT[j * DH:(j + 1) * DH, hp, qsl],
                                                  u[j][0:DH, :])
                        if hp == 0 and sq > 0:
                            phase3(sq - 1)
                    # normalize: r = 1/sums, broadcast, scale ctx (off ACT)
                    r = p2s.tile([1, HL, SQC], FP32, tag="r")
                    nc.vector.reciprocal_approx_fast(r[:], sums[:])
                    rb = p2s.tile([128, HL, SQC], FP32, tag="rb")
                    nc.gpsimd.partition_broadcast(rb[:], r[:])
                    for hp in range(2):
                        for j in range(2):
                            nc.vector.tensor_mul(
                                ctxT[j * DH:(j + 1) * DH, hp, qsl],
                                ctxT[j * DH:(j + 1) * DH, hp, qsl],
                                rb[j * DH:(j + 1) * DH, 2 * hp + j, :])
                phase3(NSQ - 1)

    nc.compile()
    return nc


def _tile_x(xT):
    # (D, S) -> (NPC, 128, PD, CW); [s4, p, c, j] = xT[c*128+p, s4*CW+j]
    return np.ascontiguousarray(
        xT.reshape(PD, 128, NPC, CW).transpose(2, 1, 0, 3).astype(ml_dtypes.bfloat16))


def kernel(query, key, value, mask, Wq, bq, Wk, bk, Wv, bv, Wo, bo, **_):
    if "nc" not in _CACHE:
        _CACHE["nc"] = _build()
    nc = _CACHE["nc"]

    query = np.asarray(query, np.float32)
    key = np.asarray(key, np.float32)
    value = np.asarray(value, np.float32)
    mask = np.asarray(mask)
    Wq = np.asarray(Wq, np.float32)
    Wk = np.asarray(Wk, np.float32)
    Wv = np.asarray(Wv, np.float32)
    Wo = np.asarray(Wo, np.float32)
    bq = np.asarray(bq, np.float32)
    bk = np.asarray(bk, np.float32)
    bv = np.asarray(bv, np.float32)
    bo = np.asarray(bo, np.float32)

    xT = {}
    keepT = {}
    for b in range(B):
        xT[b] = (
            _tile_x(query[b].T),
            _tile_x(key[b].T),
            _tile_x(value[b].T),
        )
        kp = (~mask[b]).T.astype(np.float16)  # (sk, sq)
        keepT[b] = np.ascontiguousarray(
            kp.reshape(SKT, 128, NSQ, SQC).transpose(2, 1, 0, 3))

    wsl = {}
    for g in range(GROUPS):
        fs = slice(g * F, (g + 1) * F)
        bq2 = bq[fs].reshape(2, 128).T  # [p, fc]
        bk2 = bk[fs].reshape(2, 128).T
        wsl[g] = (
            np.ascontiguousarray(
                Wq[:, fs].reshape(PD, 128, F).transpose(1, 0, 2)
            ).astype(ml_dtypes.bfloat16),
            np.ascontiguousarray(
                Wk[:, fs].reshape(PD, 128, F).transpose(1, 0, 2)
            ).astype(ml_dtypes.bfloat16),
            np.ascontiguousarray(
                Wv[:, fs].reshape(PD, 128, F).transpose(1, 0, 2)
            ).astype(ml_dtypes.bfloat16),
            np.ascontiguousarray(
                Wo[fs, :].astype(np.float16).reshape(2, 128, D).transpose(1, 0, 2)),
            np.ascontiguousarray(
                np.stack([bq2, bk2], axis=1)[:, :, :, None].astype(np.float32)),
            np.ascontiguousarray(bv[fs].reshape(1, F)),
        )

    in_maps = []
    for c in range(NCORES):
        b, g = c // GROUPS, c % GROUPS
        wq_s, wk_s, wv_s, wo_s, bqk_s, bv_s = wsl[g]
        in_maps.append({
            "xqT": xT[b][0], "xkT": xT[b][1], "xvT": xT[b][2],
            "keepT": keepT[b],
            "Wq": wq_s, "Wk": wk_s, "Wv": wv_s, "Wo": wo_s,
            "bqk": bqk_s, "bv": bv_s,
        })

    res = run_bass_kernel_spmd(nc, in_maps, core_ids=list(range(NCORES)))
    outs = [r["out"] for r in res.results]
    full = np.empty((B, S, D), np.float32)
    for b in range(B):
        acc = outs[GROUPS * b].astype(np.float32)
        for g in range(1, GROUPS):
            acc = acc + outs[GROUPS * b + g]
        full[b] = acc + bo
    return full


# revision 5
# speedup vs baseline: 1.3204x; 1.0356x over previous
"""Multi-head attention (B=2, S=2048, D=1024, H=16) on 8 NeuronCores.

Sharding: core c handles batch b = c//4 and head-group g = c%4 (4 heads,
F = 256 features). Data-parallel over B, tensor-parallel over heads:
Wq/Wk/Wv column-sliced, Wo row-sliced; host sums the 8 partial outputs.

Host pre-tiles every tensor into the exact SBUF layout so each DMA reads
large contiguous blocks per partition.

Device kernel (per core), everything transposed so no on-chip transposes:
  phase 1: qT/kT feature-major bf16 matmuls (bias fused into the PSUM
           evacuation via tensor_scalar), v s-major with a ones-column
           per head (softmax denominator rides in the AV matmul), fp16.
  phase 2 (sq-chunk outer, head-pair inner): scores^T tiles (sk, sq) via
           paired K=64 bf16 matmuls (tile_position row groups), E =
           exp(s/8) (ACT, fp16 out) * keep (DVE fp16 2x), U' = [v,1]^T @ E
           accumulated over sk in PSUM. Per chunk: denominators ->
           reciprocal_approx_fast (DVE), partition_broadcast + ctx
           normalize on GpSimd, then the output projection for that chunk
           streams out immediately (fp16 partials; host sums in fp32).
"""

import numpy as np
import ml_dtypes

import concourse.tile as tile
from concourse import bacc, mybir
from concourse.bass_utils import run_bass_kernel_spmd

B, S, D, H = 2, 2048, 1024, 16
DH = D // H  # 64
NCORES = 8
GROUPS = 4  # head groups (cores per batch)
HL = H // GROUPS  # 4 heads per core
F = HL * DH  # 256 local features
SQC = 512  # sq chunk width
NSQ = S // SQC  # 4
SKT = S // 128  # 16 sk tiles
PD = D // 128  # 8 contraction chunks
CW = 512  # phase-1 s-chunk width
NPC = S // CW

FP32 = mybir.dt.float32
BF16 = mybir.dt.bfloat16
FP16 = mybir.dt.float16

MULT = mybir.AluOpType.mult
ADD = mybir.AluOpType.add

_CACHE = {}


def _build():
    nc = bacc.Bacc("TRN2", target_bir_lowering=False, debug=False)

    xq_d = nc.dram_tensor("xqT", [NPC, 128, PD, CW], BF16, kind="ExternalInput").ap()
    xk_d = nc.dram_tensor("xkT", [NPC, 128, PD, CW], BF16, kind="ExternalInput").ap()
    xv_d = nc.dram_tensor("xvT", [NPC, 128, PD, CW], BF16, kind="ExternalInput").ap()
    keep_d = nc.dram_tensor("keepT", [NSQ, 128, SKT, SQC], BF16, kind="ExternalInput").ap()
    wq_d = nc.dram_tensor("Wq", [128, PD, F], BF16, kind="ExternalInput").ap()
    wk_d = nc.dram_tensor("Wk", [128, PD, F], BF16, kind="ExternalInput").ap()
    wv_d = nc.dram_tensor("Wv", [128, PD, F], BF16, kind="ExternalInput").ap()
    wo_d = nc.dram_tensor("Wo", [128, 2, D], BF16, kind="ExternalInput").ap()
    bqk_d = nc.dram_tensor("bqk", [128, 2, 2, 1], FP32, kind="ExternalInput").ap()
    bv_d = nc.dram_tensor("bv", [1, F], FP32, kind="ExternalInput").ap()
    out_d = nc.dram_tensor("out", [S, D], FP16, kind="ExternalOutput").ap()

    Exp = mybir.ActivationFunctionType.Exp

    with tile.TileContext(nc) as tc:
        with tc.tile_pool(name="persist", bufs=1) as pp, \
             tc.tile_pool(name="p2k", bufs=2) as p2k:
            qT = pp.tile([128, 2, S], BF16, tag="qT")  # 2 f-chunks (=head pairs)
            kT = pp.tile([128, 2, S], BF16, tag="kT")
            v2 = pp.tile([128, SKT, HL, DH + 1], BF16, tag="v2")
            ctxT = pp.tile([128, 2, S], BF16, tag="ctxT")
            wo = pp.tile([128, 2, D], BF16, tag="wo")
            bqk = pp.tile([128, 2, 2, 1], FP32, tag="bqk")
            bv_row = pp.tile([1, F], FP32, tag="bvrow")
            bv_bc = pp.tile([128, F], FP32, tag="bvbc")

            def issue_keep(sq):
                t = p2k.tile([128, SKT, SQC], BF16, tag="keep", name=f"keep_{sq}")
                nc.gpsimd.dma_start(out=t[:], in_=keep_d[sq])
                return t

            nc.sync.dma_start(out=bqk[:], in_=bqk_d)
            nc.sync.dma_start(out=bv_row[:], in_=bv_d)
            nc.gpsimd.partition_broadcast(bv_bc[:], bv_row[:])
            nc.vector.memset(v2[:, :, :, DH:DH + 1], 1.0)

            # ---- phase 1: projections ----
            with tc.tile_pool(name="p1w", bufs=1) as p1w, \
                 tc.tile_pool(name="p1x", bufs=2) as p1x, \
                 tc.tile_pool(name="ps1", bufs=2, space="PSUM") as ps1:
                wq = p1w.tile([128, PD, F], BF16, tag="wq")
                wk = p1w.tile([128, PD, F], BF16, tag="wk")
                wv = p1w.tile([128, PD, F], BF16, tag="wv")
                nc.sync.dma_start(out=wq[:], in_=wq_d)
                nc.scalar.dma_start(out=wk[:], in_=wk_d)
                nc.gpsimd.dma_start(out=wv[:], in_=wv_d)
                keep_bufs = [None, None]
                for s4 in range(NPC):
                    sl = slice(s4 * CW, (s4 + 1) * CW)
                    xq_sl = p1x.tile([128, PD, CW], BF16, tag="xq")
                    xk_sl = p1x.tile([128, PD, CW], BF16, tag="xk")
                    xv_sl = p1x.tile([128, PD, CW], BF16, tag="xv")
                    nc.sync.dma_start(out=xq_sl[:], in_=xq_d[s4])
                    nc.scalar.dma_start(out=xk_sl[:], in_=xk_d[s4])
                    nc.gpsimd.dma_start(out=xv_sl[:], in_=xv_d[s4])
                    if s4 == 0:
                        keep_bufs[0] = issue_keep(0)
                    elif s4 == 1:
                        keep_bufs[1] = issue_keep(1)
                        nc.scalar.dma_start(out=wo[:], in_=wo_d)

                    for fc in range(2):
                        fsl = slice(fc * 128, (fc + 1) * 128)
                        q_ps = ps1.tile([128, CW], FP32, tag="q_ps")
                        for d in range(PD):
                            nc.tensor.matmul(q_ps[:], wq[:, d, fsl], xq_sl[:, d, :],
                                             start=(d == 0), stop=(d == PD - 1))
                        nc.vector.tensor_scalar_add(qT[:, fc, sl], q_ps[:],
                                                    bqk[:, 0, fc, :])

                        k_ps = ps1.tile([128, CW], FP32, tag="k_ps")
                        for d in range(PD):
                            nc.tensor.matmul(k_ps[:], wk[:, d, fsl], xk_sl[:, d, :],
                                             start=(d == 0), stop=(d == PD - 1))
                        nc.vector.tensor_scalar_add(kT[:, fc, sl], k_ps[:],
                                                    bqk[:, 1, fc, :])

                    for m in range(CW // 128):  # s-subtiles of 128
                        ti = s4 * (CW // 128) + m
                        msl = slice(m * 128, (m + 1) * 128)
                        v_ps = ps1.tile([128, F], FP32, tag="v_ps")
                        for d in range(PD):
                            nc.tensor.matmul(v_ps[:], xv_sl[:, d, msl], wv[:, d, :],
                                             start=(d == 0), stop=(d == PD - 1))
                        nc.vector.scalar_tensor_tensor(
                            out=v2[:, ti, :, 0:DH],
                            in0=v_ps.rearrange("p (h e) -> p h e", h=HL),
                            scalar=1.0,
                            in1=bv_bc.rearrange("p (h e) -> p h e", h=HL),
                            op0=MULT, op1=ADD,
                        )

            # ---- phase 2+3: attention + output projection, per sq chunk ----
            # Phase 3 for chunk c is emitted inside chunk c+1 (after its hp=0
            # pass) so the in-order engine queues never stall on the
            # normalize chain at a chunk boundary.
            with tc.tile_pool(name="p2e", bufs=6) as p2e, \
                 tc.tile_pool(name="p2s", bufs=2) as p2s, \
                 tc.tile_pool(name="p3o", bufs=3) as p3o, \
                 tc.tile_pool(name="ps_st", bufs=2, space="PSUM") as ps_st, \
                 tc.tile_pool(name="ps_u", bufs=1, space="PSUM") as ps_u, \
                 tc.tile_pool(name="ps_o", bufs=2, space="PSUM") as ps_o:

                def phase3(sq):
                    for m in range(SQC // 128):
                        ti = sq * (SQC // 128) + m
                        tsl = slice(ti * 128, (ti + 1) * 128)
                        o_sb = p3o.tile([128, 2, 512], FP16, tag="o_sb",
                                        name=f"o_sb_{ti}")
                        for n in range(2):
                            nsl = slice(n * 512, (n + 1) * 512)
                            o_ps = ps_o.tile([128, 512], FP32, tag="o_ps",
                                             name=f"o_ps_{ti}_{n}")
                            for fc in range(2):
                                nc.tensor.matmul(o_ps[:], ctxT[:, fc, tsl],
                                                 wo[:, fc, nsl],
                                                 start=(fc == 0), stop=(fc == 1))
                            if n == 0:
                                nc.vector.tensor_copy(o_sb[:, n, :], o_ps[:])
                            else:
                                nc.scalar.copy(o_sb[:, n, :], o_ps[:])
                        eng = nc.sync if m % 2 == 0 else nc.scalar
                        eng.dma_start(out=out_d[tsl, :],
                                      in_=o_sb[:].rearrange("p a b -> p (a b)"))

                for sq in range(NSQ):
                    qsl = slice(sq * SQC, (sq + 1) * SQC)
                    keep = keep_bufs[sq % 2]
                    if sq + 2 < NSQ:
                        keep_bufs[sq % 2] = issue_keep(sq + 2)
                    sums = p2s.tile([1, HL, SQC], FP32, tag="sums")
                    for hp in range(2):
                        u = [ps_u.tile([128, SQC], FP32, tag=f"u{j}",
                                       name=f"u_{sq}_{hp}_{j}")
                             for j in range(2)]
                        for sk in range(SKT):
                            ksl = slice(sk * 128, (sk + 1) * 128)
                            st = ps_st.tile([128, 2, SQC], FP32, tag="st")
                            nc.tensor.matmul(st[:, 0, :], kT[0:64, hp, ksl],
                                             qT[0:64, hp, qsl], start=True, stop=True,
                                             tile_position=(0, 0))
                            nc.tensor.matmul(st[:, 1, :], kT[64:128, hp, ksl],
                                             qT[64:128, hp, qsl], start=True, stop=True,
                                             tile_position=(64, 0))
                            e_sb = p2e.tile([128, 2, SQC], BF16, tag="e")
                            nc.scalar.activation(e_sb[:], st[:], Exp, scale=0.125)
                            e2 = p2e.tile([128, 2, SQC], BF16, tag="e2")
                            nc.vector.tensor_mul(
                                e2[:], e_sb[:],
                                keep[:, sk, :].unsqueeze(1).broadcast_to(
                                    (128, 2, SQC)))
                            for j in range(2):
                                nc.tensor.matmul(
                                    u[j][0:DH + 1, :],
                                    v2[:, sk, 2 * hp + j, :],
                                    e2[:, j, :],
                                    start=(sk == 0), stop=(sk == SKT - 1),
                                )
                        for j in range(2):
                            nc.vector.tensor_copy(sums[0:1, 2 * hp + j, :],
                                                  u[j][DH:DH + 1, :])
                            nc.vector.tensor_copy(ctxT[j * DH:(j + 1) * DH, hp, qsl],
                                                  u[j][0:DH, :])
                        if hp == 0 and sq > 0:
                            phase3(sq - 1)
                    # normalize: r = 1/sums, broadcast, scale ctx (off ACT)
                    r = p2s.tile([1, HL, SQC], FP32, tag="r")
                    nc.vector.reciprocal_approx_fast(r[:], sums[:])
                    rb = p2s.tile([128, HL, SQC], FP32, tag="rb")
                    nc.gpsimd.partition_broadcast(rb[:], r[:])
                    for hp in range(2):
                        for j in range(2):
                            nc.vector.tensor_mul(
                                ctxT[j * DH:(j + 1) * DH, hp, qsl],
                                ctxT[j * DH:(j + 1) * DH, hp, qsl],
                                rb[j * DH:(j + 1) * DH, 2 * hp + j, :])
                phase3(NSQ - 1)

    nc.compile()
    return nc


def _tile_x(xT):
    # (D, S) -> (NPC, 128, PD, CW); [s4, p, c, j] = xT[c*128+p, s4*CW+j]
    return np.ascontiguousarray(
        xT.reshape(PD, 128, NPC, CW).transpose(2, 1, 0, 3).astype(ml_dtypes.bfloat16))


def kernel(query, key, value, mask, Wq, bq, Wk, bk, Wv, bv, Wo, bo, **_):
    if "nc" not in _CACHE:
        _CACHE["nc"] = _build()
    nc = _CACHE["nc"]

    query = np.asarray(query, np.float32)
    key = np.asarray(key, np.float32)
    value = np.asarray(value, np.float32)
    mask = np.asarray(mask)
    Wq = np.asarray(Wq, np.float32)
    Wk = np.asarray(Wk, np.float32)
    Wv = np.asarray(Wv, np.float32)
    Wo = np.asarray(Wo, np.float32)
    bq = np.asarray(bq, np.float32)
    bk = np.asarray(bk, np.float32)
    bv = np.asarray(bv, np.float32)
    bo = np.asarray(bo, np.float32)

    xT = {}
    keepT = {}
    for b in range(B):
        xT[b] = (
            _tile_x(query[b].T),
            _tile_x(key[b].T),
            _tile_x(value[b].T),
        )
        kp = (~mask[b]).T.astype(ml_dtypes.bfloat16)  # (sk, sq)
        keepT[b] = np.ascontiguousarray(
            kp.reshape(SKT, 128, NSQ, SQC).transpose(2, 1, 0, 3))

    wsl = {}
    for g in range(GROUPS):
        fs = slice(g * F, (g + 1) * F)
        bq2 = bq[fs].reshape(2, 128).T  # [p, fc]
        bk2 = bk[fs].reshape(2, 128).T
        wsl[g] = (
            np.ascontiguousarray(
                Wq[:, fs].reshape(PD, 128, F).transpose(1, 0, 2)
            ).astype(ml_dtypes.bfloat16),
            np.ascontiguousarray(
                Wk[:, fs].reshape(PD, 128, F).transpose(1, 0, 2)
            ).astype(ml_dtypes.bfloat16),
            np.ascontiguousarray(
                Wv[:, fs].reshape(PD, 128, F).transpose(1, 0, 2)
            ).astype(ml_dtypes.bfloat16),
            np.ascontiguousarray(
                Wo[fs, :].astype(ml_dtypes.bfloat16).reshape(2, 128, D).transpose(1, 0, 2)),
            np.ascontiguousarray(
                np.stack([bq2, bk2], axis=1)[:, :, :, None].astype(np.float32)),
            np.ascontiguousarray(bv[fs].reshape(1, F)),
        )

    in_maps = []
    for c in range(NCORES):
        b, g = c // GROUPS, c % GROUPS
        wq_s, wk_s, wv_s, wo_s, bqk_s, bv_s = wsl[g]
        in_maps.append({
            "xqT": xT[b][0], "xkT": xT[b][1], "xvT": xT[b][2],
            "keepT": keepT[b],
            "Wq": wq_s, "Wk": wk_s, "Wv": wv_s, "Wo": wo_s,
            "bqk": bqk_s, "bv": bv_s,
        })

    res = run_bass_kernel_spmd(nc, in_maps, core_ids=list(range(NCORES)))
    outs = [r["out"] for r in res.results]
    full = np.empty((B, S, D), np.float32)
    for b in range(B):
        acc = outs[GROUPS * b].astype(np.float32)
        for g in range(1, GROUPS):
            acc = acc + outs[GROUPS * b + g]
        full[b] = acc + bo
    return full


# revision 6
# speedup vs baseline: 1.3517x; 1.0237x over previous
"""Multi-head attention (B=2, S=2048, D=1024, H=16) on 8 NeuronCores.

Sharding: core c handles batch b = c//4 and head-group g = c%4 (4 heads,
F = 256 features). Data-parallel over B, tensor-parallel over heads:
Wq/Wk/Wv column-sliced, Wo row-sliced; host sums the 8 partial outputs.

Host pre-tiles every tensor into the exact SBUF layout so each DMA reads
large contiguous blocks per partition.

Device kernel (per core), everything transposed so no on-chip transposes:
  phase 1: qT/kT feature-major bf16 matmuls (bias fused into the PSUM
           evacuation via tensor_scalar), v s-major with a ones-column
           per head (softmax denominator rides in the AV matmul), fp16.
  phase 2 (sq-chunk outer, head-pair inner): scores^T tiles (sk, sq) via
           paired K=64 bf16 matmuls (tile_position row groups), E =
           exp(s/8) (ACT, fp16 out) * keep (DVE fp16 2x), U' = [v,1]^T @ E
           accumulated over sk in PSUM. Per chunk: denominators ->
           reciprocal_approx_fast (DVE), partition_broadcast + ctx
           normalize on GpSimd, then the output projection for that chunk
           streams out immediately (fp16 partials; host sums in fp32).
"""

import numpy as np
import ml_dtypes

import concourse.tile as tile
from concourse import bacc, mybir
from concourse.bass_utils import run_bass_kernel_spmd

B, S, D, H = 2, 2048, 1024, 16
DH = D // H  # 64
NCORES = 8
GROUPS = 4  # head groups (cores per batch)
HL = H // GROUPS  # 4 heads per core
F = HL * DH  # 256 local features
SQC = 512  # sq chunk width
NSQ = S // SQC  # 4
SKT = S // 128  # 16 sk tiles
PD = D // 128  # 8 contraction chunks
CW = 512  # phase-1 s-chunk width
NPC = S // CW

FP32 = mybir.dt.float32
BF16 = mybir.dt.bfloat16
FP16 = mybir.dt.float16

MULT = mybir.AluOpType.mult
ADD = mybir.AluOpType.add

_CACHE = {}


def _build():
    nc = bacc.Bacc("TRN2", target_bir_lowering=False, debug=False)

    xq_d = nc.dram_tensor("xqT", [NPC, 128, PD, CW], BF16, kind="ExternalInput").ap()
    xk_d = nc.dram_tensor("xkT", [NPC, 128, PD, CW], BF16, kind="ExternalInput").ap()
    xv_d = nc.dram_tensor("xvT", [NPC, 128, PD, CW], BF16, kind="ExternalInput").ap()
    keep_d = nc.dram_tensor("keepT", [NSQ, 128, SKT, SQC], BF16, kind="ExternalInput").ap()
    wq_d = nc.dram_tensor("Wq", [128, PD, F], BF16, kind="ExternalInput").ap()
    wk_d = nc.dram_tensor("Wk", [128, PD, F], BF16, kind="ExternalInput").ap()
    wv_d = nc.dram_tensor("Wv", [128, PD, F], BF16, kind="ExternalInput").ap()
    wo_d = nc.dram_tensor("Wo", [128, 2, D], BF16, kind="ExternalInput").ap()
    bqk_d = nc.dram_tensor("bqk", [128, 2, 2, 1], FP32, kind="ExternalInput").ap()
    bv_d = nc.dram_tensor("bv", [1, F], FP32, kind="ExternalInput").ap()
    out_d = nc.dram_tensor("out", [S, D], FP16, kind="ExternalOutput").ap()

    Exp = mybir.ActivationFunctionType.Exp

    with tile.TileContext(nc) as tc:
        with tc.tile_pool(name="persist", bufs=1) as pp, \
             tc.tile_pool(name="p2k", bufs=2) as p2k:
            qT = pp.tile([128, 2, S], BF16, tag="qT")  # 2 f-chunks (=head pairs)
            kT = pp.tile([128, 2, S], BF16, tag="kT")
            v2 = pp.tile([128, SKT, HL, 128], BF16, tag="v2")
            ctxT = pp.tile([128, 2, S], BF16, tag="ctxT")
            wo = pp.tile([128, 2, D], BF16, tag="wo")
            bqk = pp.tile([128, 2, 2, 1], FP32, tag="bqk")
            bv_row = pp.tile([1, F], FP32, tag="bvrow")
            bv_bc = pp.tile([128, F], FP32, tag="bvbc")

            def issue_keep(sq):
                t = p2k.tile([128, SKT, SQC], BF16, tag="keep", name=f"keep_{sq}")
                nc.gpsimd.dma_start(out=t[:], in_=keep_d[sq])
                return t

            nc.sync.dma_start(out=bqk[:], in_=bqk_d)
            nc.sync.dma_start(out=bv_row[:], in_=bv_d)
            nc.gpsimd.partition_broadcast(bv_bc[:], bv_row[:])
            nc.vector.memset(v2[:], 0.0)
            nc.vector.memset(v2[:, :, :, DH:DH + 1], 1.0)

            # ---- phase 1: projections ----
            with tc.tile_pool(name="p1w", bufs=1) as p1w, \
                 tc.tile_pool(name="p1x", bufs=2) as p1x, \
                 tc.tile_pool(name="ps1", bufs=2, space="PSUM") as ps1:
                wq = p1w.tile([128, PD, F], BF16, tag="wq")
                wk = p1w.tile([128, PD, F], BF16, tag="wk")
                wv = p1w.tile([128, PD, F], BF16, tag="wv")
                nc.sync.dma_start(out=wq[:], in_=wq_d)
                nc.scalar.dma_start(out=wk[:], in_=wk_d)
                nc.gpsimd.dma_start(out=wv[:], in_=wv_d)
                keep_bufs = [None, None]
                for s4 in range(NPC):
                    sl = slice(s4 * CW, (s4 + 1) * CW)
                    xq_sl = p1x.tile([128, PD, CW], BF16, tag="xq")
                    xk_sl = p1x.tile([128, PD, CW], BF16, tag="xk")
                    xv_sl = p1x.tile([128, PD, CW], BF16, tag="xv")
                    nc.sync.dma_start(out=xq_sl[:], in_=xq_d[s4])
                    nc.scalar.dma_start(out=xk_sl[:], in_=xk_d[s4])
                    nc.gpsimd.dma_start(out=xv_sl[:], in_=xv_d[s4])
                    if s4 == 0:
                        keep_bufs[0] = issue_keep(0)
                    elif s4 == 1:
                        keep_bufs[1] = issue_keep(1)
                        nc.scalar.dma_start(out=wo[:], in_=wo_d)

                    for fc in range(2):
                        fsl = slice(fc * 128, (fc + 1) * 128)
                        q_ps = ps1.tile([128, CW], FP32, tag="q_ps")
                        for d in range(PD):
                            nc.tensor.matmul(q_ps[:], wq[:, d, fsl], xq_sl[:, d, :],
                                             start=(d == 0), stop=(d == PD - 1))
                        nc.vector.tensor_scalar_add(qT[:, fc, sl], q_ps[:],
                                                    bqk[:, 0, fc, :])

                        k_ps = ps1.tile([128, CW], FP32, tag="k_ps")
                        for d in range(PD):
                            nc.tensor.matmul(k_ps[:], wk[:, d, fsl], xk_sl[:, d, :],
                                             start=(d == 0), stop=(d == PD - 1))
                        nc.vector.tensor_scalar_add(kT[:, fc, sl], k_ps[:],
                                                    bqk[:, 1, fc, :])

                    for m in range(CW // 128):  # s-subtiles of 128
                        ti = s4 * (CW // 128) + m
                        msl = slice(m * 128, (m + 1) * 128)
                        v_ps = ps1.tile([128, F], FP32, tag="v_ps")
                        for d in range(PD):
                            nc.tensor.matmul(v_ps[:], xv_sl[:, d, msl], wv[:, d, :],
                                             start=(d == 0), stop=(d == PD - 1))
                        nc.vector.scalar_tensor_tensor(
                            out=v2[:, ti, :, 0:DH],
                            in0=v_ps.rearrange("p (h e) -> p h e", h=HL),
                            scalar=1.0,
                            in1=bv_bc.rearrange("p (h e) -> p h e", h=HL),
                            op0=MULT, op1=ADD,
                        )

            # ---- phase 2+3: attention + output projection, per sq chunk ----
            # Phase 3 for chunk c is emitted inside chunk c+1 (after its hp=0
            # pass) so the in-order engine queues never stall on the
            # normalize chain at a chunk boundary.
            with tc.tile_pool(name="p2e", bufs=6) as p2e, \
                 tc.tile_pool(name="p2s", bufs=2) as p2s, \
                 tc.tile_pool(name="p3o", bufs=3) as p3o, \
                 tc.tile_pool(name="ps_st", bufs=2, space="PSUM") as ps_st, \
                 tc.tile_pool(name="ps_u", bufs=2, space="PSUM") as ps_u:

                def norm_phase3(sq, rb):
                    # ctx normalize (deferred so rb is long ready), then the
                    # output projection; o_ps rides the ps_u bank rings.
                    qsl = slice(sq * SQC, (sq + 1) * SQC)
                    for hp in range(2):
                        for j in range(2):
                            nc.vector.tensor_mul(
                                ctxT[j * DH:(j + 1) * DH, hp, qsl],
                                ctxT[j * DH:(j + 1) * DH, hp, qsl],
                                rb[j * DH:(j + 1) * DH, 2 * hp + j, :])
                    for m in range(SQC // 128):
                        ti = sq * (SQC // 128) + m
                        tsl = slice(ti * 128, (ti + 1) * 128)
                        o_sb = p3o.tile([128, 2, 512], FP16, tag="o_sb",
                                        name=f"o_sb_{ti}")
                        for n in range(2):
                            nsl = slice(n * 512, (n + 1) * 512)
                            o_ps = ps_u.tile([128, SQC], FP32, tag=f"u{n}",
                                             name=f"o_ps_{ti}_{n}")
                            for fc in range(2):
                                nc.tensor.matmul(o_ps[:], ctxT[:, fc, tsl],
                                                 wo[:, fc, nsl],
                                                 start=(fc == 0), stop=(fc == 1))
                            if n == 0:
                                nc.vector.tensor_copy(o_sb[:, n, :], o_ps[:])
                            else:
                                nc.scalar.copy(o_sb[:, n, :], o_ps[:])
                        eng = nc.sync if m % 2 == 0 else nc.scalar
                        eng.dma_start(out=out_d[tsl, :],
                                      in_=o_sb[:].rearrange("p a b -> p (a b)"))

                pending = None
                for sq in range(NSQ):
                    qsl = slice(sq * SQC, (sq + 1) * SQC)
                    keep = keep_bufs[sq % 2]
                    if sq + 2 < NSQ:
                        keep_bufs[sq % 2] = issue_keep(sq + 2)
                    sums = p2s.tile([1, HL, SQC], FP32, tag="sums")
                    for hp in range(2):
                        u = [ps_u.tile([128, SQC], FP32, tag=f"u{j}",
                                       name=f"u_{sq}_{hp}_{j}")
                             for j in range(2)]
                        for sk in range(SKT):
                            ksl = slice(sk * 128, (sk + 1) * 128)
                            st = ps_st.tile([128, 2, SQC], FP32, tag="st")
                            nc.tensor.matmul(st[:, 0, :], kT[0:64, hp, ksl],
                                             qT[0:64, hp, qsl], start=True, stop=True,
                                             tile_position=(0, 0))
                            nc.tensor.matmul(st[:, 1, :], kT[64:128, hp, ksl],
                                             qT[64:128, hp, qsl], start=True, stop=True,
                                             tile_position=(64, 0))
                            e_sb = p2e.tile([128, 2, SQC], BF16, tag="e")
                            nc.scalar.activation(e_sb[:], st[:], Exp, scale=0.125)
                            e2 = p2e.tile([128, 2, SQC], BF16, tag="e2")
                            nc.vector.tensor_mul(
                                e2[:], e_sb[:],
                                keep[:, sk, :].unsqueeze(1).broadcast_to(
                                    (128, 2, SQC)))
                            for j in range(2):
                                nc.tensor.matmul(
                                    u[j][:],
                                    v2[:, sk, 2 * hp + j, :],
                                    e2[:, j, :],
                                    start=(sk == 0), stop=(sk == SKT - 1),
                                )
                        for j in range(2):
                            nc.vector.tensor_copy(sums[0:1, 2 * hp + j, :],
                                                  u[j][DH:DH + 1, :])
                            nc.vector.tensor_copy(ctxT[j * DH:(j + 1) * DH, hp, qsl],
                                                  u[j][0:DH, :])
                        if hp == 0 and pending is not None:
                            norm_phase3(*pending)
                            pending = None
                    # r = 1/sums on DVE; broadcast on GpSimd; the ctx scaling
                    # and output projection are deferred into the next chunk.
                    r = p2s.tile([1, HL, SQC], FP32, tag="r")
                    nc.vector.reciprocal_approx_fast(r[:], sums[:])
                    rb = p2s.tile([128, HL, SQC], FP32, tag="rb")
                    nc.gpsimd.partition_broadcast(rb[:], r[:])
                    pending = (sq, rb)
                norm_phase3(*pending)

    nc.compile()
    return nc


def _tile_x(xT):
    # (D, S) -> (NPC, 128, PD, CW); [s4, p, c, j] = xT[c*128+p, s4*CW+j]
    return np.ascontiguousarray(
        xT.reshape(PD, 128, NPC, CW).transpose(2, 1, 0, 3).astype(ml_dtypes.bfloat16))


def kernel(query, key, value, mask, Wq, bq, Wk, bk, Wv, bv, Wo, bo, **_):
    if "nc" not in _CACHE:
        _CACHE["nc"] = _build()
    nc = _CACHE["nc"]

    query = np.asarray(query, np.float32)
    key = np.asarray(key, np.float32)
    value = np.asarray(value, np.float32)
    mask = np.asarray(mask)
    Wq = np.asarray(Wq, np.float32)
    Wk = np.asarray(Wk, np.float32)
    Wv = np.asarray(Wv, np.float32)
    Wo = np.asarray(Wo, np.float32)
    bq = np.asarray(bq, np.float32)
    bk = np.asarray(bk, np.float32)
    bv = np.asarray(bv, np.float32)
    bo = np.asarray(bo, np.float32)

    xT = {}
    keepT = {}
    for b in range(B):
        xT[b] = (
            _tile_x(query[b].T),
            _tile_x(key[b].T),
            _tile_x(value[b].T),
        )
        kp = (~mask[b]).T.astype(ml_dtypes.bfloat16)  # (sk, sq)
        keepT[b] = np.ascontiguousarray(
            kp.reshape(SKT, 128, NSQ, SQC).transpose(2, 1, 0, 3))

    wsl = {}
    for g in range(GROUPS):
        fs = slice(g * F, (g + 1) * F)
        bq2 = bq[fs].reshape(2, 128).T  # [p, fc]
        bk2 = bk[fs].reshape(2, 128).T
        wsl[g] = (
            np.ascontiguousarray(
                Wq[:, fs].reshape(PD, 128, F).transpose(1, 0, 2)
            ).astype(ml_dtypes.bfloat16),
            np.ascontiguousarray(
                Wk[:, fs].reshape(PD, 128, F).transpose(1, 0, 2)
            ).astype(ml_dtypes.bfloat16),
            np.ascontiguousarray(
                Wv[:, fs].reshape(PD, 128, F).transpose(1, 0, 2)
            ).astype(ml_dtypes.bfloat16),
            np.ascontiguousarray(
                Wo[fs, :].astype(ml_dtypes.bfloat16).reshape(2, 128, D).transpose(1, 0, 2)),
            np.ascontiguousarray(
                np.stack([bq2, bk2], axis=1)[:, :, :, None].astype(np.float32)),
            np.ascontiguousarray(bv[fs].reshape(1, F)),
        )

    in_maps = []
    for c in range(NCORES):
        b, g = c // GROUPS, c % GROUPS
        wq_s, wk_s, wv_s, wo_s, bqk_s, bv_s = wsl[g]
        in_maps.append({
            "xqT": xT[b][0], "xkT": xT[b][1], "xvT": xT[b][2],
            "keepT": keepT[b],
            "Wq": wq_s, "Wk": wk_s, "Wv": wv_s, "Wo": wo_s,
            "bqk": bqk_s, "bv": bv_s,
        })

    res = run_bass_kernel_spmd(nc, in_maps, core_ids=list(range(NCORES)))
    outs = [r["out"] for r in res.results]
    full = np.empty((B, S, D), np.float32)
    for b in range(B):
        acc = outs[GROUPS * b].astype(np.float32)
        for g in range(1, GROUPS):
            acc = acc + outs[GROUPS * b + g]
        full[b] = acc + bo
    return full


# revision 20
# speedup vs baseline: 1.3581x; 1.0047x over previous
"""Multi-head attention (B=2, S=2048, D=1024, H=16) on 8 NeuronCores.

Sharding: core c handles batch b = c//4 and head-group g = c%4 (4 heads,
F = 256 features). Data-parallel over B, tensor-parallel over heads:
Wq/Wk/Wv column-sliced, Wo row-sliced; host sums the 8 partial outputs.

Host pre-tiles every tensor into the exact SBUF layout so each DMA reads
large contiguous blocks per partition.

Device kernel (per core), all bf16 matmul operands, everything transposed
so no on-chip transposes:
  phase 1: qT/kT feature-major bf16 matmuls (bias fused into the PSUM
           evacuation via tensor_scalar), v s-major with a ones-column per
           head (softmax denominator rides along in the AV matmul; the
           stationary is zero-padded to 128 columns for fast weight load).
  phase 2 (sq-chunk outer, head-pair inner): scores^T (sk, sq) via
           paired K=64 matmuls (tile_position row groups) into PSUM,
           E = exp(s/8) (ACT, bf16 out), E *= keep (DVE bf16 2x, j-pair
           merged via a broadcast AP), U' = [v|1|0..]^T @ E accumulated
           over sk in double-buffered u-bank rings.
  per chunk: denominators -> reciprocal_approx_fast (DVE),
           partition_broadcast (GpSimd, single ucode library); the ctx
           normalize + output projection are deferred into the next
           chunk's instruction stream (phase-3 PSUM rides the u-bank
           rings) and stream out as fp16 partials; host sums in fp32.
"""

import numpy as np
import ml_dtypes

import concourse.tile as tile
from concourse import bacc, mybir
from concourse.bass_utils import run_bass_kernel_spmd

B, S, D, H = 2, 2048, 1024, 16
DH = D // H  # 64
NCORES = 8
GROUPS = 4  # head groups (cores per batch)
HL = H // GROUPS  # 4 heads per core
F = HL * DH  # 256 local features
SQC = 512  # sq chunk width
NSQ = S // SQC  # 4
SKT = S // 128  # 16 sk tiles
PD = D // 128  # 8 contraction chunks
CW = 512  # phase-1 s-chunk width
NPC = S // CW

FP32 = mybir.dt.float32
BF16 = mybir.dt.bfloat16
FP16 = mybir.dt.float16
FP8 = mybir.dt.float8e4

MULT = mybir.AluOpType.mult
ADD = mybir.AluOpType.add

_CACHE = {}


def _build():
    nc = bacc.Bacc("TRN2", target_bir_lowering=False, debug=False)

    xq_d = nc.dram_tensor("xqT", [NPC, 128, PD, CW], BF16, kind="ExternalInput").ap()
    xk_d = nc.dram_tensor("xkT", [NPC, 128, PD, CW], BF16, kind="ExternalInput").ap()
    xv_d = nc.dram_tensor("xvT", [NPC, 128, PD, CW], BF16, kind="ExternalInput").ap()
    keep_d = nc.dram_tensor("keepT", [NSQ, 128, SKT, SQC], BF16, kind="ExternalInput").ap()
    wq_d = nc.dram_tensor("Wq", [128, PD, F], BF16, kind="ExternalInput").ap()
    wk_d = nc.dram_tensor("Wk", [128, PD, F], BF16, kind="ExternalInput").ap()
    wv_d = nc.dram_tensor("Wv", [128, PD, F], BF16, kind="ExternalInput").ap()
    wo_d = nc.dram_tensor("Wo", [128, 2, D], BF16, kind="ExternalInput").ap()
    bqk_d = nc.dram_tensor("bqk", [128, 2, 2, 1], FP32, kind="ExternalInput").ap()
    bv_d = nc.dram_tensor("bv", [1, F], FP32, kind="ExternalInput").ap()
    out_d = nc.dram_tensor("out", [S, D], FP16, kind="ExternalOutput").ap()

    Exp = mybir.ActivationFunctionType.Exp

    with tile.TileContext(nc) as tc:
        with tc.tile_pool(name="persist", bufs=1) as pp, \
             tc.tile_pool(name="p2k", bufs=2) as p2k:
            qT = pp.tile([128, 2, S], BF16, tag="qT")  # 2 f-chunks (=head pairs)
            kT = pp.tile([128, 2, S], BF16, tag="kT")
            v2 = pp.tile([128, SKT, HL, 128], BF16, tag="v2")
            ctxT = pp.tile([128, 2, S], BF16, tag="ctxT")
            wo = pp.tile([128, 2, D], BF16, tag="wo")
            bqk = pp.tile([128, 2, 2, 1], FP32, tag="bqk")
            bv_row = pp.tile([1, F], FP32, tag="bvrow")
            bv_bc = pp.tile([128, F], FP32, tag="bvbc")

            def issue_keep(sq, eng=None):
                t = p2k.tile([128, SKT, SQC], BF16, tag="keep", name=f"keep_{sq}")
                (eng or nc.gpsimd).dma_start(out=t[:], in_=keep_d[sq])
                return t

            nc.sync.dma_start(out=bqk[:], in_=bqk_d)
            nc.sync.dma_start(out=bv_row[:], in_=bv_d)
            nc.gpsimd.partition_broadcast(bv_bc[:], bv_row[:])
            nc.vector.memset(v2[:], 0.0)
            nc.vector.memset(v2[:, :, :, DH:DH + 1], 1.0)

            # ---- phase 1: projections ----
            with tc.tile_pool(name="p1w", bufs=1) as p1w, \
                 tc.tile_pool(name="p1x", bufs=2) as p1x, \
                 tc.tile_pool(name="ps1", bufs=2, space="PSUM") as ps1:
                wq = p1w.tile([128, PD, F], BF16, tag="wq")
                wk = p1w.tile([128, PD, F], BF16, tag="wk")
                wv = p1w.tile([128, PD, F], BF16, tag="wv")
                nc.sync.dma_start(out=wq[:], in_=wq_d)
                nc.scalar.dma_start(out=wk[:], in_=wk_d)
                nc.gpsimd.dma_start(out=wv[:], in_=wv_d)
                keep_bufs = [None, None]
                for s4 in range(NPC):
                    sl = slice(s4 * CW, (s4 + 1) * CW)
                    xq_sl = p1x.tile([128, PD, CW], BF16, tag="xq")
                    xk_sl = p1x.tile([128, PD, CW], BF16, tag="xk")
                    xv_sl = p1x.tile([128, PD, CW], BF16, tag="xv")
                    nc.sync.dma_start(out=xq_sl[:], in_=xq_d[s4])
                    nc.scalar.dma_start(out=xk_sl[:], in_=xk_d[s4])
                    nc.gpsimd.dma_start(out=xv_sl[:], in_=xv_d[s4])
                    if s4 == 1:
                        keep_bufs[0] = issue_keep(0, nc.sync)
                    elif s4 == 2:
                        keep_bufs[1] = issue_keep(1, nc.sync)
                    elif s4 == 3:
                        nc.sync.dma_start(out=wo[:], in_=wo_d)

                    for fc in range(2):
                        fsl = slice(fc * 128, (fc + 1) * 128)
                        q_ps = ps1.tile([128, CW], FP32, tag="q_ps")
                        for d in range(PD):
                            nc.tensor.matmul(q_ps[:], wq[:, d, fsl], xq_sl[:, d, :],
                                             start=(d == 0), stop=(d == PD - 1))
                        nc.vector.tensor_scalar_add(qT[:, fc, sl], q_ps[:],
                                                    bqk[:, 0, fc, :])

                        k_ps = ps1.tile([128, CW], FP32, tag="k_ps")
                        for d in range(PD):
                            nc.tensor.matmul(k_ps[:], wk[:, d, fsl], xk_sl[:, d, :],
                                             start=(d == 0), stop=(d == PD - 1))
                        nc.vector.tensor_scalar_add(kT[:, fc, sl], k_ps[:],
                                                    bqk[:, 1, fc, :])

                    for m in range(CW // 128):  # s-subtiles of 128
                        ti = s4 * (CW // 128) + m
                        msl = slice(m * 128, (m + 1) * 128)
                        v_ps = ps1.tile([128, F], FP32, tag="v_ps")
                        for d in range(PD):
                            nc.tensor.matmul(v_ps[:], xv_sl[:, d, msl], wv[:, d, :],
                                             start=(d == 0), stop=(d == PD - 1))
                        nc.vector.scalar_tensor_tensor(
                            out=v2[:, ti, :, 0:DH],
                            in0=v_ps.rearrange("p (h e) -> p h e", h=HL),
                            scalar=1.0,
                            in1=bv_bc.rearrange("p (h e) -> p h e", h=HL),
                            op0=MULT, op1=ADD,
                        )

            # ---- phase 2+3: attention + output projection, per sq chunk ----
            # Phase 3 for chunk c is emitted inside chunk c+1 (after its hp=0
            # pass) so the in-order engine queues never stall on the
            # normalize chain at a chunk boundary.
            with tc.tile_pool(name="p2e", bufs=6) as p2e, \
                 tc.tile_pool(name="p2s", bufs=2) as p2s, \
                 tc.tile_pool(name="p3o", bufs=3) as p3o, \
                 tc.tile_pool(name="ps_st", bufs=2, space="PSUM") as ps_st, \
                 tc.tile_pool(name="ps_u", bufs=2, space="PSUM") as ps_u:

                def norm_phase3(sq, rb):
                    # ctx normalize (deferred so rb is long ready), then the
                    # output projection; o_ps rides the ps_u bank rings.
                    qsl = slice(sq * SQC, (sq + 1) * SQC)
                    for hp in range(2):
                        for j in range(2):
                            nc.vector.tensor_mul(
                                ctxT[j * DH:(j + 1) * DH, hp, qsl],
                                ctxT[j * DH:(j + 1) * DH, hp, qsl],
                                rb[j * DH:(j + 1) * DH, 2 * hp + j, :])
                    for m in range(SQC // 128):
                        ti = sq * (SQC // 128) + m
                        tsl = slice(ti * 128, (ti + 1) * 128)
                        o_sb = p3o.tile([128, 2, 512], FP16, tag="o_sb",
                                        name=f"o_sb_{ti}")
                        for n in range(2):
                            nsl = slice(n * 512, (n + 1) * 512)
                            o_ps = ps_u.tile([128, SQC], FP32, tag=f"u{n}",
                                             name=f"o_ps_{ti}_{n}")
                            for fc in range(2):
                                nc.tensor.matmul(o_ps[:], ctxT[:, fc, tsl],
                                                 wo[:, fc, nsl],
                                                 start=(fc == 0), stop=(fc == 1))
                            if n == 0:
                                nc.vector.tensor_copy(o_sb[:, n, :], o_ps[:])
                            else:
                                nc.scalar.copy(o_sb[:, n, :], o_ps[:])
                        eng = nc.sync if m % 2 == 0 else nc.scalar
                        eng.dma_start(out=out_d[tsl, :],
                                      in_=o_sb[:].rearrange("p a b -> p (a b)"))

                def emit_scores(sq, hp, sk):
                    qsl = slice(sq * SQC, (sq + 1) * SQC)
                    ksl = slice(sk * 128, (sk + 1) * 128)
                    st = ps_st.tile([128, 2, SQC], FP32, tag="st",
                                    name=f"st_{sq}_{hp}_{sk}")
                    nc.tensor.matmul(st[:, 0, :], kT[0:64, hp, ksl],
                                     qT[0:64, hp, qsl], start=True, stop=True,
                                     tile_position=(0, 0))
                    nc.tensor.matmul(st[:, 1, :], kT[64:128, hp, ksl],
                                     qT[64:128, hp, qsl], start=True, stop=True,
                                     tile_position=(64, 0))
                    return st

                pending = None
                st_hold = None
                for sq in range(NSQ):
                    qsl = slice(sq * SQC, (sq + 1) * SQC)
                    keep = keep_bufs[sq % 2]
                    if sq + 2 < NSQ:
                        keep_bufs[sq % 2] = issue_keep(sq + 2)
                    sums = p2s.tile([1, HL, SQC], FP32, tag="sums")
                    for hp in range(2):
                        u = [ps_u.tile([128, SQC], FP32, tag=f"u{j}",
                                       name=f"u_{sq}_{hp}_{j}")
                             for j in range(2)]
                        for sk in range(SKT):
                            if sk == 0 and st_hold is not None:
                                st = st_hold
                                st_hold = None
                            else:
                                st = emit_scores(sq, hp, sk)
                            if sk == SKT - 1 and (hp, sq) != (1, NSQ - 1):
                                nhp, nsq = (hp + 1, sq) if hp == 0 else (0, sq + 1)
                                st_hold = emit_scores(nsq, nhp, 0)
                            e_sb = p2e.tile([128, 2, SQC], BF16, tag="e")
                            nc.scalar.activation(e_sb[:], st[:], Exp, scale=0.125)
                            e2 = p2e.tile([128, 2, SQC], BF16, tag="e2")
                            nc.vector.tensor_mul(
                                e2[:], e_sb[:],
                                keep[:, sk, :].unsqueeze(1).broadcast_to(
                                    (128, 2, SQC)))
                            for j in range(2):
                                nc.tensor.matmul(
                                    u[j][:],
                                    v2[:, sk, 2 * hp + j, :],
                                    e2[:, j, :],
                                    start=(sk == 0), stop=(sk == SKT - 1),
                                )
                        for j in range(2):
                            nc.vector.tensor_copy(sums[0:1, 2 * hp + j, :],
                                                  u[j][DH:DH + 1, :])
                            nc.vector.tensor_copy(ctxT[j * DH:(j + 1) * DH, hp, qsl],
                                                  u[j][0:DH, :])
                        if hp == 0 and pending is not None:
                            norm_phase3(*pending)
                            pending = None
                    # r = 1/sums on DVE; broadcast on GpSimd; the ctx scaling
                    # and output projection are deferred into the next chunk.
                    r = p2s.tile([1, HL, SQC], FP32, tag="r")
                    nc.vector.reciprocal_approx_fast(r[:], sums[:])
                    rb = p2s.tile([128, HL, SQC], FP32, tag="rb")
                    nc.gpsimd.partition_broadcast(rb[:], r[:])
                    pending = (sq, rb)
                norm_phase3(*pending)

    nc.compile()
    return nc


def _tile_x(xT):
    # (D, S) -> (NPC, 128, PD, CW); [s4, p, c, j] = xT[c*128+p, s4*CW+j]
    return np.ascontiguousarray(
        xT.reshape(PD, 128, NPC, CW).transpose(2, 1, 0, 3).astype(ml_dtypes.bfloat16))


def kernel(query, key, value, mask, Wq, bq, Wk, bk, Wv, bv, Wo, bo, **_):
    if "nc" not in _CACHE:
        _CACHE["nc"] = _build()
    nc = _CACHE["nc"]

    query = np.asarray(query, np.float32)
    key = np.asarray(key, np.float32)
    value = np.asarray(value, np.float32)
    mask = np.asarray(mask)
    Wq = np.asarray(Wq, np.float32)
    Wk = np.asarray(Wk, np.float32)
    Wv = np.asarray(Wv, np.float32)
    Wo = np.asarray(Wo, np.float32)
    bq = np.asarray(bq, np.float32)
    bk = np.asarray(bk, np.float32)
    bv = np.asarray(bv, np.float32)
    bo = np.asarray(bo, np.float32)

    xT = {}
    keepT = {}
    for b in range(B):
        xT[b] = (
            _tile_x(query[b].T),
            _tile_x(key[b].T),
            _tile_x(value[b].T),
        )
        kp = (~mask[b]).T.astype(ml_dtypes.bfloat16)  # (sk, sq)
        keepT[b] = np.ascontiguousarray(
            kp.reshape(SKT, 128, NSQ, SQC).transpose(2, 1, 0, 3))

    wsl = {}
    for g in range(GROUPS):
        fs = slice(g * F, (g + 1) * F)
        bq2 = bq[fs].reshape(2, 128).T  # [p, fc]
        bk2 = bk[fs].reshape(2, 128).T
        wsl[g] = (
            np.ascontiguousarray(
                Wq[:, fs].reshape(PD, 128, F).transpose(1, 0, 2)
            ).astype(ml_dtypes.bfloat16),
            np.ascontiguousarray(
                Wk[:, fs].reshape(PD, 128, F).transpose(1, 0, 2)
            ).astype(ml_dtypes.bfloat16),
            np.ascontiguousarray(
                Wv[:, fs].reshape(PD, 128, F).transpose(1, 0, 2)
            ).astype(ml_dtypes.bfloat16),
            np.ascontiguousarray(
                Wo[fs, :].astype(ml_dtypes.bfloat16).reshape(2, 128, D).transpose(1, 0, 2)),
            np.ascontiguousarray(
                np.stack([bq2, bk2], axis=1)[:, :, :, None].astype(np.float32)),
            np.ascontiguousarray(bv[fs].reshape(1, F)),
        )

    in_maps = []
    for c in range(NCORES):
        b, g = c // GROUPS, c % GROUPS
        wq_s, wk_s, wv_s, wo_s, bqk_s, bv_s = wsl[g]
        in_maps.append({
            "xqT": xT[b][0], "xkT": xT[b][1], "xvT": xT[b][2],
            "keepT": keepT[b],
            "Wq": wq_s, "Wk": wk_s, "Wv": wv_s, "Wo": wo_s,
            "bqk": bqk_s, "bv": bv_s,
        })

    res = run_bass_kernel_spmd(nc, in_maps, core_ids=list(range(NCORES)))
    outs = [r["out"] for r in res.results]
    full = np.empty((B, S, D), np.float32)
    for b in range(B):
        acc = outs[GROUPS * b].astype(np.float32)
        for g in range(1, GROUPS):
            acc = acc + outs[GROUPS * b + g]
        full[b] = acc + bo
    return full


# revision 22
# speedup vs baseline: 1.3772x; 1.0141x over previous
"""Multi-head attention (B=2, S=2048, D=1024, H=16) on 8 NeuronCores.

Sharding: core c handles batch b = c//4 and head-group g = c%4 (4 heads,
F = 256 features). Data-parallel over B, tensor-parallel over heads:
Wq/Wk/Wv column-sliced, Wo row-sliced; host sums the 8 partial outputs.

Host pre-tiles every tensor into the exact SBUF layout so each DMA reads
large contiguous blocks per partition.

Device kernel (per core), all bf16 matmul operands, everything transposed
so no on-chip transposes:
  phase 1: qT/kT feature-major bf16 matmuls (bias fused into the PSUM
           evacuation via tensor_scalar), v s-major with a ones-column per
           head (softmax denominator rides along in the AV matmul; the
           stationary is zero-padded to 128 columns for fast weight load).
  phase 2 (sq-chunk outer, head-pair inner): scores^T (sk, sq) via
           paired K=64 matmuls (tile_position row groups) into PSUM,
           E = exp(s/8) (ACT, bf16 out), E *= keep (DVE bf16 2x, j-pair
           merged via a broadcast AP), U' = [v|1|0..]^T @ E accumulated
           over sk in double-buffered u-bank rings.
  per chunk: denominators -> reciprocal_approx_fast (DVE),
           partition_broadcast (GpSimd, single ucode library); the ctx
           normalize + output projection are deferred into the next
           chunk's instruction stream (phase-3 PSUM rides the u-bank
           rings) and stream out as fp16 partials; host sums in fp32.
"""

import numpy as np
import ml_dtypes

import concourse.tile as tile
from concourse import bacc, mybir
from concourse.bass_utils import run_bass_kernel_spmd

B, S, D, H = 2, 2048, 1024, 16
DH = D // H  # 64
NCORES = 8
GROUPS = 4  # head groups (cores per batch)
HL = H // GROUPS  # 4 heads per core
F = HL * DH  # 256 local features
SQC = 512  # sq chunk width
NSQ = S // SQC  # 4
SKT = S // 128  # 16 sk tiles
PD = D // 128  # 8 contraction chunks
CW = 512  # phase-1 s-chunk width
NPC = S // CW

FP32 = mybir.dt.float32
BF16 = mybir.dt.bfloat16
FP16 = mybir.dt.float16
FP8 = mybir.dt.float8e4

MULT = mybir.AluOpType.mult
ADD = mybir.AluOpType.add

_CACHE = {}


def _build():
    nc = bacc.Bacc("TRN2", target_bir_lowering=False, debug=False)

    xq_d = nc.dram_tensor("xqT", [NPC, 128, PD, CW], BF16, kind="ExternalInput").ap()
    xk_d = nc.dram_tensor("xkT", [NPC, 128, PD, CW], BF16, kind="ExternalInput").ap()
    xv_d = nc.dram_tensor("xvT", [NPC, 128, PD, CW], BF16, kind="ExternalInput").ap()
    keep_d = nc.dram_tensor("keepT", [NSQ, 128, SKT, SQC], BF16, kind="ExternalInput").ap()
    wq_d = nc.dram_tensor("Wq", [128, PD, F], BF16, kind="ExternalInput").ap()
    wk_d = nc.dram_tensor("Wk", [128, PD, F], BF16, kind="ExternalInput").ap()
    wv_d = nc.dram_tensor("Wv", [128, PD, F], BF16, kind="ExternalInput").ap()
    wo_d = nc.dram_tensor("Wo", [128, 2, D], BF16, kind="ExternalInput").ap()
    bqk_d = nc.dram_tensor("bqk", [128, 2, 2, 1], FP32, kind="ExternalInput").ap()
    bv_d = nc.dram_tensor("bv", [1, F], FP32, kind="ExternalInput").ap()
    out_d = nc.dram_tensor("out", [S, D], FP16, kind="ExternalOutput").ap()

    Exp = mybir.ActivationFunctionType.Exp

    with tile.TileContext(nc) as tc:
        with tc.tile_pool(name="persist", bufs=1) as pp, \
             tc.tile_pool(name="p2k", bufs=2) as p2k:
            qT = pp.tile([128, 2, S], BF16, tag="qT")  # 2 f-chunks (=head pairs)
            kT = pp.tile([128, 2, S], BF16, tag="kT")
            v2 = pp.tile([128, SKT, HL, 128], BF16, tag="v2")
            ctxT = pp.tile([128, 2, S], BF16, tag="ctxT")
            wo = pp.tile([128, 2, D], BF16, tag="wo")
            bqk = pp.tile([128, 2, 2, 1], FP32, tag="bqk")
            bv_row = pp.tile([1, F], FP32, tag="bvrow")
            bv_bc = pp.tile([128, F], FP32, tag="bvbc")

            def issue_keep(sq, eng=None):
                t = p2k.tile([128, SKT, SQC], BF16, tag="keep", name=f"keep_{sq}")
                (eng or nc.gpsimd).dma_start(out=t[:], in_=keep_d[sq])
                return t

            nc.sync.dma_start(out=bqk[:], in_=bqk_d)
            nc.sync.dma_start(out=bv_row[:], in_=bv_d)
            nc.gpsimd.partition_broadcast(bv_bc[:], bv_row[:])
            nc.vector.memset(v2[:], 0.0)
            nc.vector.memset(v2[:, :, :, DH:DH + 1], 1.0)

            # ---- phase 1: projections ----
            with tc.tile_pool(name="p1w", bufs=1) as p1w, \
                 tc.tile_pool(name="p1x", bufs=2) as p1x, \
                 tc.tile_pool(name="ps1", bufs=2, space="PSUM") as ps1:
                wq = p1w.tile([128, PD, F], BF16, tag="wq")
                wk = p1w.tile([128, PD, F], BF16, tag="wk")
                wv = p1w.tile([128, PD, F], BF16, tag="wv")
                nc.sync.dma_start(out=wq[:], in_=wq_d)
                nc.scalar.dma_start(out=wk[:], in_=wk_d)
                nc.gpsimd.dma_start(out=wv[:], in_=wv_d)
                keep_bufs = [None, None]
                for s4 in range(NPC):
                    sl = slice(s4 * CW, (s4 + 1) * CW)
                    xq_sl = p1x.tile([128, PD, CW], BF16, tag="xq")
                    xk_sl = p1x.tile([128, PD, CW], BF16, tag="xk")
                    xv_sl = p1x.tile([128, PD, CW], BF16, tag="xv")
                    nc.sync.dma_start(out=xq_sl[:], in_=xq_d[s4])
                    nc.scalar.dma_start(out=xk_sl[:], in_=xk_d[s4])
                    nc.gpsimd.dma_start(out=xv_sl[:], in_=xv_d[s4])
                    if s4 == 1:
                        keep_bufs[0] = issue_keep(0, nc.sync)
                    elif s4 == 2:
                        keep_bufs[1] = issue_keep(1, nc.sync)
                    elif s4 == 3:
                        nc.sync.dma_start(out=wo[:], in_=wo_d)

                    for fc in range(2):
                        fsl = slice(fc * 128, (fc + 1) * 128)
                        q_ps = ps1.tile([128, CW], FP32, tag="q_ps")
                        for d in range(PD):
                            nc.tensor.matmul(q_ps[:], wq[:, d, fsl], xq_sl[:, d, :],
                                             start=(d == 0), stop=(d == PD - 1))
                        nc.vector.tensor_scalar_add(qT[:, fc, sl], q_ps[:],
                                                    bqk[:, 0, fc, :])

                        k_ps = ps1.tile([128, CW], FP32, tag="k_ps")
                        for d in range(PD):
                            nc.tensor.matmul(k_ps[:], wk[:, d, fsl], xk_sl[:, d, :],
                                             start=(d == 0), stop=(d == PD - 1))
                        nc.vector.tensor_scalar_add(kT[:, fc, sl], k_ps[:],
                                                    bqk[:, 1, fc, :])

                    for m in range(CW // 128):  # s-subtiles of 128
                        ti = s4 * (CW // 128) + m
                        msl = slice(m * 128, (m + 1) * 128)
                        v_ps = ps1.tile([128, F], FP32, tag="v_ps")
                        for d in range(PD):
                            nc.tensor.matmul(v_ps[:], xv_sl[:, d, msl], wv[:, d, :],
                                             start=(d == 0), stop=(d == PD - 1))
                        nc.vector.scalar_tensor_tensor(
                            out=v2[:, ti, :, 0:DH],
                            in0=v_ps.rearrange("p (h e) -> p h e", h=HL),
                            scalar=1.0,
                            in1=bv_bc.rearrange("p (h e) -> p h e", h=HL),
                            op0=MULT, op1=ADD,
                        )

            # ---- phase 2+3: attention + output projection, per sq chunk ----
            # Phase 3 for chunk c is emitted inside chunk c+1 (after its hp=0
            # pass) so the in-order engine queues never stall on the
            # normalize chain at a chunk boundary.
            with tc.tile_pool(name="p2e", bufs=6) as p2e, \
                 tc.tile_pool(name="p2s", bufs=2) as p2s, \
                 tc.tile_pool(name="p3o", bufs=3) as p3o, \
                 tc.tile_pool(name="ps_st", bufs=2, space="PSUM") as ps_st, \
                 tc.tile_pool(name="ps_u", bufs=2, space="PSUM") as ps_u:

                def norm_phase3(sq, rb):
                    # ctx normalize (deferred so rb is long ready), then the
                    # output projection; o_ps rides the ps_u bank rings.
                    qsl = slice(sq * SQC, (sq + 1) * SQC)
                    for hp in range(2):
                        for j in range(2):
                            nc.vector.tensor_mul(
                                ctxT[j * DH:(j + 1) * DH, hp, qsl],
                                ctxT[j * DH:(j + 1) * DH, hp, qsl],
                                rb[j * DH:(j + 1) * DH, 2 * hp + j, :])
                    for m in range(SQC // 128):
                        ti = sq * (SQC // 128) + m
                        tsl = slice(ti * 128, (ti + 1) * 128)
                        o_sb = p3o.tile([128, 2, 512], FP16, tag="o_sb",
                                        name=f"o_sb_{ti}")
                        for n in range(2):
                            nsl = slice(n * 512, (n + 1) * 512)
                            o_ps = ps_u.tile([128, SQC], FP32, tag=f"u{n}",
                                             name=f"o_ps_{ti}_{n}")
                            for fc in range(2):
                                nc.tensor.matmul(o_ps[:], ctxT[:, fc, tsl],
                                                 wo[:, fc, nsl],
                                                 start=(fc == 0), stop=(fc == 1))
                            if n == 0:
                                nc.vector.tensor_copy(o_sb[:, n, :], o_ps[:])
                            else:
                                nc.scalar.copy(o_sb[:, n, :], o_ps[:])
                        eng = nc.sync if m % 2 == 0 else nc.scalar
                        eng.dma_start(out=out_d[tsl, :],
                                      in_=o_sb[:].rearrange("p a b -> p (a b)"))

                def emit_scores(sq, hp, sk):
                    qsl = slice(sq * SQC, (sq + 1) * SQC)
                    ksl = slice(sk * 128, (sk + 1) * 128)
                    st = ps_st.tile([128, 2, SQC], FP32, tag="st",
                                    name=f"st_{sq}_{hp}_{sk}")
                    nc.tensor.matmul(st[:, 0, :], kT[0:64, hp, ksl],
                                     qT[0:64, hp, qsl], start=True, stop=True,
                                     tile_position=(0, 0))
                    nc.tensor.matmul(st[:, 1, :], kT[64:128, hp, ksl],
                                     qT[64:128, hp, qsl], start=True, stop=True,
                                     tile_position=(64, 0))
                    return st

                pending = None
                st_hold = None
                for sq in range(NSQ):
                    qsl = slice(sq * SQC, (sq + 1) * SQC)
                    keep = keep_bufs[sq % 2]
                    if sq + 2 < NSQ:
                        keep_bufs[sq % 2] = issue_keep(sq + 2)
                    sums = p2s.tile([1, HL, SQC], FP32, tag="sums")
                    for hp in range(2):
                        u = [ps_u.tile([128, SQC], FP32, tag=f"u{j}",
                                       name=f"u_{sq}_{hp}_{j}")
                             for j in range(2)]
                        for sk in range(SKT):
                            if sk == 0 and st_hold is not None:
                                st = st_hold
                                st_hold = None
                            else:
                                st = emit_scores(sq, hp, sk)
                            if sk == SKT - 1 and (hp, sq) != (1, NSQ - 1):
                                nhp, nsq = (hp + 1, sq) if hp == 0 else (0, sq + 1)
                                st_hold = emit_scores(nsq, nhp, 0)
                            e_sb = p2e.tile([128, 2, SQC], BF16, tag="e")
                            nc.scalar.activation(e_sb[:], st[:], Exp, scale=0.125)
                            e2 = p2e.tile([128, 2, SQC], BF16, tag="e2")
                            nc.vector.tensor_mul(
                                e2[:], e_sb[:],
                                keep[:, sk, :].unsqueeze(1).broadcast_to(
                                    (128, 2, SQC)))
                            for j in range(2):
                                nc.tensor.matmul(
                                    u[j][:],
                                    v2[:, sk, 2 * hp + j, :],
                                    e2[:, j, :],
                                    start=(sk == 0), stop=(sk == SKT - 1),
                                )
                        for j in range(2):
                            nc.vector.tensor_copy(sums[0:1, 2 * hp + j, :],
                                                  u[j][DH:DH + 1, :])
                            nc.vector.tensor_copy(ctxT[j * DH:(j + 1) * DH, hp, qsl],
                                                  u[j][0:DH, :])
                        if hp == 0 and pending is not None:
                            norm_phase3(*pending)
                            pending = None
                    # r = 1/sums on DVE; broadcast on GpSimd; the ctx scaling
                    # and output projection are deferred into the next chunk.
                    r = p2s.tile([1, HL, SQC], FP32, tag="r")
                    nc.vector.reciprocal_approx_fast(r[:], sums[:])
                    rb = p2s.tile([128, HL, SQC], FP32, tag="rb")
                    nc.gpsimd.partition_broadcast(rb[:], r[:])
                    pending = (sq, rb)
                norm_phase3(*pending)

    nc.compile()
    return nc


def _tile_x(xT):
    # (D, S) -> (NPC, 128, PD, CW); [s4, p, c, j] = xT[c*128+p, s4*CW+j]
    return np.ascontiguousarray(
        xT.reshape(PD, 128, NPC, CW).transpose(2, 1, 0, 3).astype(ml_dtypes.bfloat16))


def kernel(query, key, value, mask, Wq, bq, Wk, bk, Wv, bv, Wo, bo, **_):
    if "nc" not in _CACHE:
        _CACHE["nc"] = _build()
    nc = _CACHE["nc"]

    query = np.asarray(query, np.float32)
    key = np.asarray(key, np.float32)
    value = np.asarray(value, np.float32)
    mask = np.asarray(mask)
    Wq = np.asarray(Wq, np.float32)
    Wk = np.asarray(Wk, np.float32)
    Wv = np.asarray(Wv, np.float32)
    Wo = np.asarray(Wo, np.float32)
    bq = np.asarray(bq, np.float32)
    bk = np.asarray(bk, np.float32)
    bv = np.asarray(bv, np.float32)
    bo = np.asarray(bo, np.float32)

    xT = {}
    keepT = {}
    for b in range(B):
        xT[b] = (
            _tile_x(query[b].T),
            _tile_x(key[b].T),
            _tile_x(value[b].T),
        )
        kp = (~mask[b]).T.astype(ml_dtypes.bfloat16)  # (sk, sq)
        keepT[b] = np.ascontiguousarray(
            kp.reshape(SKT, 128, NSQ, SQC).transpose(2, 1, 0, 3))

    wsl = {}
    for g in range(GROUPS):
        fs = slice(g * F, (g + 1) * F)
        bq2 = bq[fs].reshape(2, 128).T  # [p, fc]
        bk2 = bk[fs].reshape(2, 128).T
        wsl[g] = (
            np.ascontiguousarray(
                Wq[:, fs].reshape(PD, 128, F).transpose(1, 0, 2)
            ).astype(ml_dtypes.bfloat16),
            np.ascontiguousarray(
                Wk[:, fs].reshape(PD, 128, F).transpose(1, 0, 2)
            ).astype(ml_dtypes.bfloat16),
            np.ascontiguousarray(
                Wv[:, fs].reshape(PD, 128, F).transpose(1, 0, 2)
            ).astype(ml_dtypes.bfloat16),
            np.ascontiguousarray(
                Wo[fs, :].astype(ml_dtypes.bfloat16).reshape(2, 128, D).transpose(1, 0, 2)),
            np.ascontiguousarray(
                np.stack([bq2, bk2], axis=1)[:, :, :, None].astype(np.float32)),
            np.ascontiguousarray(bv[fs].reshape(1, F)),
        )

    in_maps = []
    for c in range(NCORES):
        b, g = c // GROUPS, c % GROUPS
        wq_s, wk_s, wv_s, wo_s, bqk_s, bv_s = wsl[g]
        in_maps.append({
            "xqT": xT[b][0], "xkT": xT[b][1], "xvT": xT[b][2],
            "keepT": keepT[b],
            "Wq": wq_s, "Wk": wk_s, "Wv": wv_s, "Wo": wo_s,
            "bqk": bqk_s, "bv": bv_s,
        })

    res = run_bass_kernel_spmd(nc, in_maps, core_ids=list(range(NCORES)))
    outs = [r["out"] for r in res.results]
    full = np.empty((B, S, D), np.float32)
    for b in range(B):
        acc = outs[GROUPS * b].astype(np.float32)
        for g in range(1, GROUPS):
            acc = acc + outs[GROUPS * b + g]
        full[b] = acc + bo
    return full
